# revision 1
# baseline (speedup 1.0000x reference)
"""GCN layer kernel for TRN2, data-parallel over batch across 8 NeuronCores.

Per core (one batch b):
  phase A: stream adjT (bf16 shadow) -> deg matvec on PE; load x, build xT via
           PE transposes.
  transition: deg -> dis -> u (col layout); z = u*x; c1/c2 row broadcast.
  phase B: agg0T[d,i] = sum_j adjT[j,i] * z[j,d] as fp32r matmuls, two half
           passes over i with 8 PSUM banks; epilogue folds the self loop:
           aggT = c1[i]*agg0T + c2[i]*xT.
  phase C: out2[l,o] = aggT.T @ W.T + b (bias via K=1 matmul), relu/scale,
           residual, layernorm via moments; stage-sliced emission (groups of
           4 row-blocks) to pipeline the strict-FIFO engines.
"""
import os
import numpy as np
import ml_dtypes

import concourse.bacc as bacc
import concourse.tile as tile
import concourse.mybir as mybir
from concourse.bass_utils import run_bass_kernel_spmd

B, L, D = 8, 2048, 512
JBN = L // 128      # 16 row blocks
NCH = L // 512      # 4 i-chunks of 512
DBN = D // 128      # 4 d-blocks
LN_EPS = 1e-5
DSCALE = float(D) ** -0.5
F32 = mybir.dt.float32
F32R = mybir.dt.float32r
BF16 = mybir.dt.bfloat16
MUL = mybir.AluOpType.mult
ADD = mybir.AluOpType.add
SUB = mybir.AluOpType.subtract

LAST_RESULT = None  # BassKernelResults of the most recent run (for profiling)


def _round_fp32r(v: np.ndarray) -> np.ndarray:
    """RNE-round fp32 to e8m11-in-top-20-bits (matches HW fp32r rounding)."""
    bits = np.ascontiguousarray(v, dtype=np.float32).view(np.uint32)
    r = bits + np.uint32(0x7FF) + ((bits >> np.uint32(12)) & np.uint32(1))
    r &= np.uint32(0xFFFFF000)
    return r.view(np.float32)


def _build_program(ln_identity=False, bias_zero=False):
    nc = bacc.Bacc("TRN2", target_bir_lowering=False, debug=False)
    d = {}
    def di(name, shape, dt):
        d[name] = nc.dram_tensor(name, shape, dt, kind="ExternalInput").ap()
    di("adjT_r", [L, L], F32R)
    di("adjT_h", [L, L], BF16)
    di("x_in", [L, D], F32)
    di("validc_f", [128, JBN], F32)
    di("validc_h", [128, JBN], BF16)
    di("ewc", [128, 1], F32)
    di("wt_r", [D, D], F32R)
    di("b_row_r", [1, D], F32R)
    di("ones_row", [1, 128], F32R)
    di("lnw_row", [1, D], F32)
    di("lnb_row", [1, D], F32)
    di("ident", [128, 128], F32)
    out_d = nc.dram_tensor("out_t", [L, D], F32, kind="ExternalOutput").ap()

    with tile.TileContext(nc) as tc:
        with tc.tile_pool(name="pX", bufs=JBN) as pX, \
             tc.tile_pool(name="pAgg", bufs=JBN) as pAgg, \
             tc.tile_pool(name="pW", bufs=DBN) as pW, \
             tc.tile_pool(name="pStat", bufs=1) as pStat, \
             tc.tile_pool(name="pCol", bufs=32) as pCol, \
             tc.tile_pool(name="pSmall", bufs=1) as pSmall:

            # ---- persistent arrays + global statics ----
            wt_t = [pW.tile([128, D], F32R, tag="wt", name=f"wt{k}")
                    for k in range(DBN)]
            eps_t = pSmall.tile([128, 1], F32, tag="eps")
            nc.vector.memset(eps_t[:], LN_EPS)
            ones_t = pSmall.tile([1, 128], F32R, tag="ones")
            nc.scalar.dma_start(ones_t[:], d["ones_row"][:])
            browr_t = pSmall.tile([1, D], F32R, tag="browr")
            nc.scalar.dma_start(browr_t[:], d["b_row_r"][:])
            x_t = [pX.tile([128, D], F32, tag="x", name=f"x{j}") for j in range(JBN)]
            agg_t = [pAgg.tile([128, D], F32R, tag="agg", name=f"agg{j}")
                     for j in range(JBN)]
            stat_b = {}

            with tc.tile_pool(name="pZ", bufs=JBN) as pZ, \
                 tc.tile_pool(name="pXT", bufs=DBN) as pXT, \
                 tc.tile_pool(name="pB", bufs=10) as pB, \
                 tc.tile_pool(name="pC", bufs=1) as pC, \
                 tc.tile_pool(name="psMM", bufs=4, space="PSUM") as psMM:
                psPT_cm = tc.tile_pool(name="psPT", bufs=2, space="PSUM")
                psPT = psPT_cm.__enter__()
                psMisc_cm = tc.tile_pool(name="psMisc", bufs=2, space="PSUM")
                psMisc = psMisc_cm.__enter__()
                z_t = [pZ.tile([128, D], F32R, tag="z", name=f"z{j}")
                       for j in range(JBN)]
                xT_t = [pXT.tile([128, L], BF16, tag="xT", name=f"xT{m}")
                        for m in range(DBN)]
                c1b = pC.tile([128, L], F32, tag="c1b")
                c2b = pC.tile([128, L], F32, tag="c2b")

                # ---- transient scope: phase A + transition ----
                with tc.tile_pool(name="pTrans", bufs=1) as pTrans, \
                     tc.tile_pool(name="pA", bufs=3) as pA:
                    ident_t = pTrans.tile([128, 128], F32, tag="ident")
                    nc.scalar.dma_start(ident_t[:], d["ident"][:])
                    validf_t = pTrans.tile([128, JBN], F32, tag="vf")
                    nc.scalar.dma_start(validf_t[:], d["validc_f"][:])
                    validh_t = pTrans.tile([128, JBN], BF16, tag="vh")
                    nc.scalar.dma_start(validh_t[:], d["validc_h"][:])
                    ewc_t = pTrans.tile([128, 1], F32, tag="ew")
                    nc.scalar.dma_start(ewc_t[:], d["ewc"][:])
                    rows = {}
                    for nm in ("lnw_row", "lnb_row"):
                        r = pTrans.tile([1, D], F32, tag=nm, name=nm + "_t")
                        nc.scalar.dma_start(r[:], d[nm][:])
                        rows[nm] = r
                    for nm in ("lnw_row", "lnb_row"):
                        t = pStat.tile([128, D], F32, tag=nm + "b", name=nm + "_b")
                        nc.gpsimd.partition_broadcast(t[:], rows[nm][:])
                        stat_b[nm] = t

                    # phase A: deg matvecs (bf16, N=1, col layout) + x load
                    # + xT build on PE
                    deg_ps = [psMisc.tile([128, 512], F32, tag="misc",
                                          name=f"deg_ps{i}") for i in range(2)]
                    for jb in range(JBN):
                        adjA = pA.tile([128, L], BF16, tag="adjA")
                        nc.sync.dma_start(
                            adjA[:], d["adjT_h"][jb * 128:(jb + 1) * 128, :])
                        for n in range(NCH):
                            po = 32 * (n % 2)
                            nc.tensor.matmul(
                                deg_ps[n // 2][po:po + 1, :],
                                validh_t[:, jb:jb + 1],
                                adjA[:, n * 512:(n + 1) * 512],
                                start=(jb == 0), stop=(jb == JBN - 1))
                        nc.scalar.dma_start(
                            x_t[jb][:], d["x_in"][jb * 128:(jb + 1) * 128, :])
                        for m in range(DBN):
                            pt = psPT.tile([128, 128], F32, tag="pt")
                            nc.tensor.transpose(
                                pt[:], x_t[jb][:, m * 128:(m + 1) * 128],
                                ident_t[:])
                            nc.vector.tensor_copy(
                                xT_t[m][:, jb * 128:(jb + 1) * 128], pt[:])
                    r_sb = pTrans.tile([128, 1024], F32, tag="rsb")
                    for n in range(NCH):
                        po = 32 * (n % 2)
                        nc.vector.tensor_copy(
                            r_sb[po:po + 1, (n // 2) * 512:(n // 2 + 1) * 512],
                            deg_ps[n // 2][po:po + 1, :])
                    rc_ps = psMisc.tile([128, JBN], F32, tag="misc", name="rc_ps")
                    for v in range(JBN):
                        n, c = v // 4, v % 4
                        po = 32 * (n % 2)
                        fo = (n // 2) * 512 + c * 128
                        nc.tensor.transpose(
                            rc_ps[:, v:v + 1],
                            r_sb[po:po + 1, fo:fo + 128],
                            ident_t[po:po + 1, po:po + 1])
                    r_col = pCol.tile([128, JBN], F32, tag="rcol", bufs=1)
                    nc.vector.tensor_copy(r_col[:], rc_ps[:])

                    deg_col = pCol.tile([128, JBN], F32, tag="degc", bufs=1)
                    nc.vector.tensor_mul(deg_col[:], r_col[:], validf_t[:])
                    nc.vector.tensor_scalar_add(deg_col[:], deg_col[:], 1.0)
                    std_col = pCol.tile([128, JBN], F32, tag="stdc", bufs=1)
                    nc.scalar.sqrt(std_col[:], deg_col[:])
                    dis_col = pCol.tile([128, JBN], F32, tag="disc", bufs=1)
                    nc.vector.reciprocal(dis_col[:], std_col[:])
                    u_col = pCol.tile([128, JBN], F32, tag="uc", bufs=1)
                    nc.vector.tensor_mul(u_col[:], dis_col[:], validf_t[:])

                    c1_col = pCol.tile([128, JBN], F32, tag="c1c", bufs=1)
                    nc.vector.tensor_scalar_mul(c1_col[:], u_col[:], ewc_t[:])
                    c2_col = pCol.tile([128, JBN], F32, tag="c2c", bufs=1)
                    nc.vector.scalar_tensor_tensor(
                        c2_col[:], dis_col[:], ewc_t[:], dis_col[:], MUL, MUL)

                    # c1/c2 -> row chunks -> one partition_broadcast per vector
                    for nm, col, bc in (("c1", c1_col, c1b), ("c2", c2_col, c2b)):
                        rcf = pTrans.tile([1, L], F32, tag="crow", bufs=1,
                                          name=f"{nm}rowf")
                        for n in range(NCH):
                            rp = psMisc.tile([1, 512], F32, tag="misc",
                                             name=f"{nm}rp{n}")
                            for q in range(4):
                                v = n * 4 + q
                                nc.tensor.transpose(
                                    rp[0:1, q * 128:(q + 1) * 128],
                                    col[:, v:v + 1], ident_t[:])
                            nc.vector.tensor_copy(rcf[:, n * 512:(n + 1) * 512],
                                                  rp[:])
                        nc.gpsimd.partition_broadcast(bc[:], rcf[:])

                    # z tiles (DVE rounds to fp32r)
                    for jb in range(JBN):
                        nc.vector.tensor_scalar_mul(z_t[jb][:], x_t[jb][:],
                                                    u_col[:, jb:jb + 1])

                # ---- close phase-A psum pools; open C-side pools ----
                psMisc_cm.__exit__(None, None, None)
                psPT_cm.__exit__(None, None, None)

                for k in range(DBN):
                    nc.scalar.dma_start(wt_t[k][:],
                                        d["wt_r"][k * 128:(k + 1) * 128, :])

                # ---- fused phases B & C: pass p feeds layernorm group p ----
                G = 4
                with tc.tile_pool(name="pScr", bufs=16) as pScr, \
                     tc.tile_pool(name="pOut", bufs=5) as pOut, \
                     tc.tile_pool(name="psC", bufs=4, space="PSUM") as psC:
                    mm_ps = {}
                    for p in range(NCH):
                        # -- pass p: MM1 quarter
                        for m in range(DBN):
                            mm_ps[(p, m)] = psMM.tile([128, 512], F32, tag="mm",
                                                      name=f"mm1_{p}_{m}")
                        for jb in range(JBN):
                            jsl = slice(jb * 128, (jb + 1) * 128)
                            adjQ = pB.tile([128, 512], F32R, tag="adjB")
                            nc.sync.dma_start(
                                adjQ[:], d["adjT_r"][jsl, p * 512:(p + 1) * 512])
                            for m in range(DBN):
                                nc.tensor.matmul(
                                    mm_ps[(p, m)][:],
                                    z_t[jb][:, m * 128:(m + 1) * 128],
                                    adjQ[:], start=(jb == 0), stop=(jb == JBN - 1))
                        # -- epilogue p: aggT = c1*agg0T + c2*xT
                        sl = slice(p * 512, (p + 1) * 512)
                        t2d = {}
                        for m in range(DBN):
                            t2 = pScr.tile([128, 512], F32, tag="scr",
                                           name=f"t2_{p}_{m}")
                            nc.vector.tensor_mul(t2[:], mm_ps[(p, m)][:],
                                                 c1b[:, sl])
                            t2d[m] = t2
                        for m in range(DBN):
                            tmp = pScr.tile([128, 512], F32, tag="scr",
                                            name=f"tp_{p}_{m}")
                            nc.gpsimd.tensor_mul(tmp[:], xT_t[m][:, sl],
                                                 c2b[:, sl])
                            nc.gpsimd.tensor_add(agg_t[m * NCH + p][:],
                                                 t2d[m][:], tmp[:])
                        # -- layernorm group p: lbs 4p..4p+3
                        lbs = list(range(G * p, G * (p + 1)))
                        ps2d, rd, hhd, sumd, m2d = {}, {}, {}, {}, {}
                        mud, rstdd, t1d = {}, {}, {}
                        for lb in lbs:
                            n, off = lb // 4, (lb % 4) * 128
                            ps2 = psC.tile([128, D], F32, tag="mmc",
                                           name=f"mm2_{lb}")
                            for k in range(DBN):
                                nc.tensor.matmul(
                                    ps2[:], agg_t[k * NCH + n][:, off:off + 128],
                                    wt_t[k][:], start=(k == 0),
                                    stop=(bias_zero and k == DBN - 1))
                            if not bias_zero:
                                nc.tensor.matmul(ps2[:], ones_t[:], browr_t[:],
                                                 start=False, stop=True)
                            ps2d[lb] = ps2
                        for lb in lbs:
                            r = pScr.tile([128, D], F32, tag="scr", name=f"r{lb}")
                            nc.scalar.activation(r[:], ps2d[lb][:],
                                                 mybir.ActivationFunctionType.Relu,
                                                 scale=DSCALE)
                            rd[lb] = r
                        for lb in lbs:
                            hh = pScr.tile([128, D], F32, tag="scr", name=f"hh{lb}")
                            sums = pCol.tile([128, 1], F32, tag="lncol",
                                             name=f"su{lb}")
                            nc.vector.scalar_tensor_tensor(
                                hh[:], rd[lb][:], 1.0, x_t[lb][:], MUL, ADD,
                                accum_out=sums[:])
                            hhd[lb], sumd[lb] = hh, sums
                        for lb in lbs:
                            sq = pScr.tile([128, D], F32, tag="scr", name=f"sq{lb}")
                            m2s = pCol.tile([128, 1], F32, tag="lncol",
                                            name=f"m2{lb}")
                            nc.vector.scalar_tensor_tensor(
                                sq[:], hhd[lb][:], 1.0, hhd[lb][:], MUL, MUL,
                                accum_out=m2s[:])
                            m2d[lb] = m2s
                        for lb in lbs:
                            mu = pCol.tile([128, 1], F32, tag="lncol",
                                           name=f"mu{lb}")
                            nc.scalar.mul(mu[:], sumd[lb][:], 1.0 / D)
                            m2n = pCol.tile([128, 1], F32, tag="lncol",
                                            name=f"mn{lb}")
                            nc.scalar.mul(m2n[:], m2d[lb][:], 1.0 / D)
                            negv = pCol.tile([128, 1], F32, tag="lncol",
                                             name=f"nv{lb}")
                            nc.vector.scalar_tensor_tensor(
                                negv[:], mu[:], mu[:], m2n[:], MUL, SUB)
                            stdt = pCol.tile([128, 1], F32, tag="lncol",
                                             name=f"sd{lb}")
                            nc.scalar.activation(
                                stdt[:], negv[:],
                                mybir.ActivationFunctionType.Sqrt,
                                scale=-1.0, bias=eps_t[:])
                            rstd = pCol.tile([128, 1], F32, tag="lncol",
                                             name=f"rs{lb}")
                            nc.vector.reciprocal(rstd[:], stdt[:])
                            mud[lb], rstdd[lb] = mu, rstd
                        for lb in lbs:
                            eng1 = nc.gpsimd if lb % 2 == 0 else nc.vector
                            t1 = pOut.tile([128, D], F32, tag="o", name=f"t1{lb}")
                            eng1.tensor_scalar(t1[:], hhd[lb][:], mud[lb][:],
                                               rstdd[lb][:], SUB, MUL)
                            t1d[lb] = t1
                        if ln_identity:
                            for lb in lbs:
                                nc.sync.dma_start(
                                    out_d[lb * 128:(lb + 1) * 128, :], t1d[lb][:])
                        else:
                            for lb in lbs:
                                tt = pScr.tile([128, D], F32, tag="scr",
                                               name=f"tt{lb}")
                                teng = nc.vector if lb % 2 == 0 else nc.gpsimd
                                teng.tensor_mul(tt[:], t1d[lb][:],
                                                stat_b["lnw_row"][:])
                                o_sb = pOut.tile([128, D], F32, tag="o",
                                                 name=f"o{lb}")
                                nc.gpsimd.tensor_add(o_sb[:], tt[:],
                                                     stat_b["lnb_row"][:])
                                nc.sync.dma_start(
                                    out_d[lb * 128:(lb + 1) * 128, :], o_sb[:])

    nc.compile()
    return nc


_NC_CACHE = {}


def _get_nc(ln_identity=False, bias_zero=False):
    key = (ln_identity, bias_zero)
    if key not in _NC_CACHE:
        _NC_CACHE[key] = _build_program(*key)
    return _NC_CACHE[key]


def kernel(x, adj, pad_mask, W, b, ln_w, ln_b, edge_weight):
    global LAST_RESULT
    x = np.asarray(x, dtype=np.float32)
    adj = np.asarray(adj, dtype=np.float32)
    pad_mask = np.asarray(pad_mask)
    W = np.asarray(W, dtype=np.float32)
    b = np.asarray(b, dtype=np.float32)
    ln_w = np.asarray(ln_w, dtype=np.float32)
    ln_b = np.asarray(ln_b, dtype=np.float32)
    ew = float(np.asarray(edge_weight).reshape(-1)[0])

    ln_identity = bool(np.all(ln_w == 1.0) and np.all(ln_b == 0.0))
    bias_zero = bool(np.all(b == 0.0))
    nc = _get_nc(ln_identity, bias_zero)

    wt_r = _round_fp32r(np.ascontiguousarray(W.T))
    ewc = np.full((128, 1), ew, dtype=np.float32)
    ident = np.eye(128, dtype=np.float32)
    b_row_r = _round_fp32r(b.reshape(1, D))
    ones_row = np.ones((1, 128), dtype=np.float32)
    lnw_row = np.ascontiguousarray(ln_w.reshape(1, D))
    lnb_row = np.ascontiguousarray(ln_b.reshape(1, D))

    in_maps = []
    for c in range(B):
        adjT = np.ascontiguousarray(adj[c].T)
        valid = (~pad_mask[c]).astype(np.float32)
        validc = np.ascontiguousarray(valid.reshape(JBN, 128).T)
        in_maps.append({
            "adjT_r": _round_fp32r(adjT),
            "adjT_h": adjT.astype(ml_dtypes.bfloat16),
            "x_in": np.ascontiguousarray(x[c]),
            "validc_f": validc,
            "validc_h": validc.astype(ml_dtypes.bfloat16),
            "ewc": ewc,
            "wt_r": wt_r,
            "b_row_r": b_row_r,
            "ones_row": ones_row,
            "lnw_row": lnw_row,
            "lnb_row": lnb_row,
            "ident": ident,
        })

    trace = os.environ.get("KERNEL_TRACE", "0") == "1"
    res = run_bass_kernel_spmd(nc, in_maps, core_ids=list(range(B)), trace=trace)
    LAST_RESULT = res
    out = np.stack([res.results[c]["out_t"] for c in range(B)], axis=0)
    return out



# revision 15
# speedup vs baseline: 1.3839x; 1.3839x over previous
"""GCN layer kernel for TRN2, data-parallel over batch across 8 NeuronCores.

Host folds the pad mask and the self-loop into the adjacency (aT = (adj *
valid_i * valid_j + I).T, cast bf16), so the device sees a single [L,L] bf16
matrix that it loads ONCE and keeps SBUF-resident:

  phase A: stream aT tiles; deg row-sums via PE matvec vs a ones column
           (4 PSUM accumulation groups over 16 row tiles). deg -> dis via
           sqrt+reciprocal in column layout; z[jb] = dis * x[jb] (bf16).
  phase B: agg0T[d,i] = sum_j aT[j,i] * z[j,d] as bf16 matmuls, four 512-wide
           i-chunks x 4 d-blocks accumulated over 16 j-tiles in PSUM.
  phase C: per i-chunk: drain PSUM -> bf16 aggT, MM2 ps2[l,o] = aggT.T @ W.T,
           relu folded with the D^-1/2 * edge_weight * dis_l scale as a
           per-partition activation scale, then residual + layernorm via
           moments, stream out.

The dis_i scale of the aggregation commutes with MM2 (it is per output row l),
so the whole normalization epilogue collapses into the relu's scale operand —
no transposed-x tiles, no c1/c2 broadcast passes.
"""
import os
import numpy as np
import ml_dtypes

import concourse.bacc as bacc
import concourse.tile as tile
import concourse.mybir as mybir
from concourse.bass_utils import run_bass_kernel_spmd

B, L, D = 8, 2048, 512
JBN = L // 128      # 16 row blocks (j tiles / l blocks)
NCH = L // 512      # 4 i-chunks of 512
DBN = D // 128      # 4 d-blocks
LN_EPS = 1e-5
DSCALE = float(D) ** -0.5
F32 = mybir.dt.float32
BF16 = mybir.dt.bfloat16
MUL = mybir.AluOpType.mult
ADD = mybir.AluOpType.add
SUB = mybir.AluOpType.subtract

LAST_RESULT = None  # BassKernelResults of the most recent run (for profiling)


def _build_program(fold_scale=True, ln_identity=True):
    """fold_scale: edge_weight >= 0 and bias == 0, so relu(c*v)*s == the
    activation with a per-partition scale. ln_identity: ln_w == 1, ln_b == 0."""
    nc = bacc.Bacc("TRN2", target_bir_lowering=False, debug=False)
    d = {}
    def di(name, shape, dt):
        d[name] = nc.dram_tensor(name, shape, dt, kind="ExternalInput").ap()
    di("ident", [128, 128], F32)
    di("aT_h", [L, L], BF16)
    di("x_in", [L, D], F32)
    di("wt_h", [D, D], BF16)
    if not fold_scale:
        di("ewc", [128, 1], F32)
        di("b_row", [1, D], F32)
    if not ln_identity:
        di("lnw_row", [1, D], F32)
        di("lnb_row", [1, D], F32)
    out_d = nc.dram_tensor("out_t", [L, D], F32, kind="ExternalOutput").ap()

    with tile.TileContext(nc) as tc:
        with tc.tile_pool(name="pA", bufs=JBN) as pA, \
             tc.tile_pool(name="pX", bufs=JBN) as pX, \
             tc.tile_pool(name="pZ", bufs=JBN) as pZ, \
             tc.tile_pool(name="pW", bufs=DBN) as pW, \
             tc.tile_pool(name="pCol", bufs=12) as pCol, \
             tc.tile_pool(name="pSmall", bufs=1) as pSmall:

            # ---- persistent arrays ----
            aT_t = [pA.tile([128, L], BF16, tag="aT", name=f"aT{j}")
                    for j in range(JBN)]
            x_t = [pX.tile([128, D], F32, tag="x", name=f"x{j}") for j in range(JBN)]
            z_t = [pZ.tile([128, D], BF16, tag="z", name=f"z{j}")
                   for j in range(JBN)]
            wt_t = [pW.tile([128, D], BF16, tag="wt", name=f"wt{k}")
                    for k in range(DBN)]
            eps_t = pSmall.tile([128, 1], F32, tag="eps")
            nc.vector.memset(eps_t[:], LN_EPS)
            stat_b = {}

            # input DMA priority order: aT tiles first (deg gates everything),
            # then ident (needed ~28us), x tiles, W
            onesc_t = pSmall.tile([128, 1], BF16, tag="onesc")
            nc.vector.memset(onesc_t[:], 1.0)
            ident_t = pSmall.tile([128, 128], F32, tag="ident")
            nc.scalar.dma_start(ident_t[:], d["ident"][:])
            # first tile split in half so the deg matvecs start sooner
            nc.sync.dma_start(aT_t[0][:, 0:1024], d["aT_h"][0:128, 0:1024])
            nc.sync.dma_start(aT_t[0][:, 1024:L], d["aT_h"][0:128, 1024:L])
            for jb in range(1, JBN):
                nc.sync.dma_start(aT_t[jb][:], d["aT_h"][jb * 128:(jb + 1) * 128, :])
            for jb in range(JBN):
                nc.sync.dma_start(x_t[jb][:], d["x_in"][jb * 128:(jb + 1) * 128, :])
            for k in range(DBN):
                nc.sync.dma_start(wt_t[k][:], d["wt_h"][k * 128:(k + 1) * 128, :])
            if not fold_scale:
                ewc_t = pSmall.tile([128, 1], F32, tag="ew")
                nc.scalar.dma_start(ewc_t[:], d["ewc"][:])
                b_r = pSmall.tile([1, D], F32, tag="brow")
                nc.scalar.dma_start(b_r[:], d["b_row"][:])
                bbT = pSmall.tile([128, D], F32, tag="bb")
                nc.gpsimd.partition_broadcast(bbT[:], b_r[:])
            if not ln_identity:
                rows = {}
                for nm in ("lnw_row", "lnb_row"):
                    r = pSmall.tile([1, D], F32, tag=nm, name=nm + "_t")
                    nc.scalar.dma_start(r[:], d[nm][:])
                    rows[nm] = r
                for nm in ("lnw_row", "lnb_row"):
                    t = pSmall.tile([128, D], F32, tag=nm + "b", name=nm + "_b")
                    nc.gpsimd.partition_broadcast(t[:], rows[nm][:])
                    stat_b[nm] = t

            # ---- phase A: deg row sums on PE as aT tiles land ----
            with tc.tile_pool(name="psDeg", bufs=2, space="PSUM") as psDeg, \
                 tc.tile_pool(name="psPT", bufs=1, space="PSUM") as psPT, \
                 tc.tile_pool(name="pTr", bufs=1) as pTr:
                deg_ps = [psDeg.tile([128, 512], F32, tag="deg",
                                     name=f"deg_ps{i}") for i in range(2)]
                for jb in range(JBN):
                    for n in range(NCH):
                        po = 32 * (n % 2)
                        nc.tensor.matmul(
                            deg_ps[n // 2][po:po + 1, :],
                            onesc_t[:],
                            aT_t[jb][:, n * 512:(n + 1) * 512],
                            start=(jb == 0), stop=(jb == JBN - 1))
                # deg rows -> column layout, two-stage pipeline: each
                # deg_ps tile is copied wide (rows 0..32 in one op), its 8
                # columns transposed, then sqrt straight off PSUM so the
                # first z tiles are ready while stage 2 still runs
                r_sb = pTr.tile([128, 1024], F32, tag="rsb")
                rc_ps = psPT.tile([128, JBN], F32, tag="rc")
                std_col = pCol.tile([128, JBN], F32, tag="stdc", bufs=1)
                dis_col = pCol.tile([128, JBN], F32, tag="disc", bufs=1)
                for t in range(2):
                    csl = slice(t * 8, t * 8 + 8)
                    nc.vector.tensor_copy(r_sb[0:33, t * 512:(t + 1) * 512],
                                          deg_ps[t][0:33, :])
                    for v in range(t * 8, t * 8 + 8):
                        n, c = v // 4, v % 4
                        po = 32 * (n % 2)
                        fo = (n // 2) * 512 + c * 128
                        nc.tensor.transpose(
                            rc_ps[:, v:v + 1],
                            r_sb[po:po + 1, fo:fo + 128],
                            ident_t[po:po + 1, po:po + 1])
                    nc.scalar.sqrt(std_col[:, csl], rc_ps[:, csl])
                    nc.vector.reciprocal(dis_col[:, csl], std_col[:, csl])
                    for jb in range(t * 8, t * 8 + 8):
                        nc.vector.tensor_scalar_mul(
                            z_t[jb][:], x_t[jb][:], dis_col[:, jb:jb + 1])

            if fold_scale:
                # relu scale: DSCALE * ew(=|ew| folded host-side) * dis_l
                c1s_col = pCol.tile([128, JBN], F32, tag="c1s", bufs=1)
                nc.scalar.mul(c1s_col[:], dis_col[:], DSCALE)
            else:
                c1_col = pCol.tile([128, JBN], F32, tag="c1c", bufs=1)
                nc.vector.tensor_scalar_mul(c1_col[:], dis_col[:], ewc_t[:])

            # ---- phases B & C fused per i-chunk ----
            with tc.tile_pool(name="psMM", bufs=6, space="PSUM") as psMM, \
                 tc.tile_pool(name="psC", bufs=2, space="PSUM") as psC, \
                 tc.tile_pool(name="pAgg", bufs=8) as pAgg, \
                 tc.tile_pool(name="pScr", bufs=14) as pScr, \
                 tc.tile_pool(name="pOut", bufs=5) as pOut:
                # last 512 split into two 256-wide chunks to shorten the
                # serial post-MM tail (drain->MM2->relu->LN->out)
                chunks = [(0, 512), (512, 512), (1024, 512),
                          (1536, 256), (1792, 256)]
                for p, (ioff, iw) in enumerate(chunks):
                    last_c = (p == len(chunks) - 1)
                    isl = slice(ioff, ioff + iw)
                    mm_ps = [psMM.tile([128, 512], F32, tag="mm",
                                       name=f"mm1_{p}_{m}") for m in range(DBN)]
                    for jb in range(JBN):
                        for m in range(DBN):
                            nc.tensor.matmul(
                                mm_ps[m][:, :iw],
                                z_t[jb][:, m * 128:(m + 1) * 128],
                                aT_t[jb][:, isl],
                                start=(jb == 0), stop=(jb == JBN - 1))
                    # drain PSUM -> bf16 aggT; only DVE/Act may read PSUM
                    agg_t = []
                    for m in range(DBN):
                        a = pAgg.tile([128, 512], BF16, tag="agg",
                                      name=f"agg_{p}_{m}")
                        if m % 2 == 1:
                            nc.scalar.copy(a[:, :iw], mm_ps[m][:, :iw])
                        else:
                            nc.vector.tensor_copy(a[:, :iw], mm_ps[m][:, :iw])
                        agg_t.append(a)
                    # MM2 + relu + layernorm for the l-blocks of chunk p
                    lbs = list(range(ioff // 128, (ioff + iw) // 128))
                    ps2d, rd, hhd, sumd, m2d = {}, {}, {}, {}, {}
                    mud, rstdd, t1d = {}, {}, {}
                    for lb in lbs:
                        off = lb * 128 - ioff
                        ps2 = psC.tile([128, D], F32, tag="mmc",
                                       name=f"mm2_{lb}")
                        for k in range(DBN):
                            nc.tensor.matmul(
                                ps2[:], agg_t[k][:, off:off + 128],
                                wt_t[k][:], start=(k == 0), stop=(k == DBN - 1))
                        ps2d[lb] = ps2
                    for lb in lbs:
                        r = pScr.tile([128, D], F32, tag="scr", name=f"r{lb}")
                        if fold_scale:
                            nc.scalar.activation(
                                r[:], ps2d[lb][:],
                                mybir.ActivationFunctionType.Relu,
                                scale=c1s_col[:, lb:lb + 1])
                        else:
                            tmp = pScr.tile([128, D], F32, tag="scr",
                                            name=f"tb{lb}")
                            nc.vector.scalar_tensor_tensor(
                                tmp[:], ps2d[lb][:], c1_col[:, lb:lb + 1],
                                bbT[:], MUL, ADD)
                            nc.scalar.activation(
                                r[:], tmp[:],
                                mybir.ActivationFunctionType.Relu,
                                scale=DSCALE)
                        rd[lb] = r
                    for lb in lbs:
                        # hh on DVE (accum ops are DVE/Act only); the square
                        # pass rides the Act engine so the chains pipeline
                        hh = pScr.tile([128, D], F32, tag="scr", name=f"hh{lb}")
                        sums = pCol.tile([128, 1], F32, tag="lncol",
                                         name=f"su{lb}")
                        nc.vector.scalar_tensor_tensor(
                            hh[:], rd[lb][:], 1.0, x_t[lb][:], MUL, ADD,
                            accum_out=sums[:])
                        hhd[lb], sumd[lb] = hh, sums
                    for lb in lbs:
                        sq = pScr.tile([128, D], F32, tag="scr", name=f"sq{lb}")
                        m2s = pCol.tile([128, 1], F32, tag="lncol",
                                        name=f"m2{lb}")
                        nc.scalar.activation(
                            sq[:], hhd[lb][:],
                            mybir.ActivationFunctionType.Square,
                            accum_out=m2s[:])
                        m2d[lb] = m2s
                    for lb in lbs:
                        mu = pCol.tile([128, 1], F32, tag="lncol",
                                       name=f"mu{lb}")
                        nc.scalar.mul(mu[:], sumd[lb][:], 1.0 / D)
                        m2n = pCol.tile([128, 1], F32, tag="lncol",
                                        name=f"mn{lb}")
                        nc.scalar.mul(m2n[:], m2d[lb][:], 1.0 / D)
                        negv = pCol.tile([128, 1], F32, tag="lncol",
                                         name=f"nv{lb}")
                        nc.vector.scalar_tensor_tensor(
                            negv[:], mu[:], mu[:], m2n[:], MUL, SUB)
                        stdt = pCol.tile([128, 1], F32, tag="lncol",
                                         name=f"sd{lb}")
                        nc.scalar.activation(
                            stdt[:], negv[:],
                            mybir.ActivationFunctionType.Sqrt,
                            scale=-1.0, bias=eps_t[:])
                        rstd = pCol.tile([128, 1], F32, tag="lncol",
                                         name=f"rs{lb}")
                        nc.vector.reciprocal(rstd[:], stdt[:])
                        mud[lb], rstdd[lb] = mu, rstd
                    for lb in lbs:
                        eng1 = nc.gpsimd if (lb % 2 == 0) != last_c else nc.vector
                        t1 = pOut.tile([128, D], F32, tag="o", name=f"t1{lb}")
                        eng1.tensor_scalar(t1[:], hhd[lb][:], mud[lb][:],
                                           rstdd[lb][:], SUB, MUL)
                        t1d[lb] = t1
                    if ln_identity:
                        for lb in lbs:
                            nc.scalar.dma_start(
                                out_d[lb * 128:(lb + 1) * 128, :], t1d[lb][:])
                    else:
                        for lb in lbs:
                            tt = pScr.tile([128, D], F32, tag="scr",
                                           name=f"tt{lb}")
                            teng = nc.vector if lb % 2 == 0 else nc.gpsimd
                            teng.tensor_mul(tt[:], t1d[lb][:],
                                            stat_b["lnw_row"][:])
                            o_sb = pOut.tile([128, D], F32, tag="o",
                                             name=f"o{lb}")
                            nc.gpsimd.tensor_add(o_sb[:], tt[:],
                                                 stat_b["lnb_row"][:])
                            nc.scalar.dma_start(
                                out_d[lb * 128:(lb + 1) * 128, :], o_sb[:])

    nc.compile()
    return nc


_NC_CACHE = {}


def _get_nc(fold_scale=True, ln_identity=True):
    key = (fold_scale, ln_identity)
    if key not in _NC_CACHE:
        _NC_CACHE[key] = _build_program(*key)
    return _NC_CACHE[key]


def kernel(x, adj, pad_mask, W, b, ln_w, ln_b, edge_weight):
    global LAST_RESULT
    x = np.asarray(x, dtype=np.float32)
    adj = np.asarray(adj, dtype=np.float32)
    pad_mask = np.asarray(pad_mask)
    W = np.asarray(W, dtype=np.float32)
    b = np.asarray(b, dtype=np.float32)
    ln_w = np.asarray(ln_w, dtype=np.float32)
    ln_b = np.asarray(ln_b, dtype=np.float32)
    ew = float(np.asarray(edge_weight).reshape(-1)[0])

    ln_identity = bool(np.all(ln_w == 1.0) and np.all(ln_b == 0.0))
    fold_scale = bool(np.all(b == 0.0) and ew >= 0.0)
    nc = _get_nc(fold_scale, ln_identity)

    ident = np.eye(128, dtype=np.float32)
    # fold_scale: ew commutes with W inside the relu argument, so fold it
    # into the weights host-side; deg/dis/z stay ew-free.
    w_eff = W.T * ew if fold_scale else W.T
    wt_h = np.ascontiguousarray(w_eff).astype(ml_dtypes.bfloat16)
    eye = np.eye(L, dtype=np.float32)

    in_maps = []
    for c in range(B):
        valid = (~pad_mask[c]).astype(np.float32)
        aT = (adj[c].T * valid[:, None]) * valid[None, :]
        aT += eye
        im = {
            "ident": ident,
            "aT_h": aT.astype(ml_dtypes.bfloat16),
            "x_in": np.ascontiguousarray(x[c]),
            "wt_h": wt_h,
        }
        if not fold_scale:
            im["ewc"] = np.full((128, 1), ew, dtype=np.float32)
            im["b_row"] = np.ascontiguousarray(b.reshape(1, D))
        if not ln_identity:
            im["lnw_row"] = np.ascontiguousarray(ln_w.reshape(1, D))
            im["lnb_row"] = np.ascontiguousarray(ln_b.reshape(1, D))
        in_maps.append(im)

    trace = os.environ.get("KERNEL_TRACE", "0") == "1"
    res = run_bass_kernel_spmd(nc, in_maps, core_ids=list(range(B)), trace=trace)
    LAST_RESULT = res
    out = np.stack([res.results[c]["out_t"] for c in range(B)], axis=0)
    return out


# revision 16
# speedup vs baseline: 1.4348x; 1.0368x over previous
"""GCN layer kernel for TRN2, data-parallel over batch across 8 NeuronCores.

v4: the two matmuls compose linearly (relu sits after BOTH), so
  out2[l,o] = dis_l * sum_j aT[j,l] * (dis_j * (x @ W.T)[j,o])
XW = x @ W.T is computed on the PE during the otherwise idle adjacency
stream window (x tiles are DMA'd first), via PE transposes of x. After deg
-> dis, y = dis * XW (bf16) becomes the moving operand of the single big
matmul with stationary aT column slices, which lands the result directly in
[l, o] layout, one PSUM bank per 128-row block: relu (with the per-partition
dis_l*DSCALE scale) reads PSUM straight, then residual + layernorm. No
second matmul pass, no PSUM->SBUF agg drains, no transposed output fixups.

Phases:
  H: DMA ident, x0, W, x1.., aT0..15. PE: per x tile, 4 transposes ->
     xT bf16, 4 matmuls -> XW psum -> bf16 SBUF; then deg matvecs ride
     the aT stream (PSUM accum over 16 j tiles).
  T: deg rows -> col layout (PE transposes), sqrt off PSUM, reciprocal;
     y[jb] = xw[jb] * dis_col[jb] on DVE/Pool alternating.
  M: 16 i-blocks x (16 j accumulation matmuls), relu+LN chain per block,
     stream out.
"""
import os
import numpy as np
import ml_dtypes

import concourse.bacc as bacc
import concourse.tile as tile
import concourse.mybir as mybir
from concourse.bass_utils import run_bass_kernel_spmd

B, L, D = 8, 2048, 512
JBN = L // 128      # 16 row blocks (j tiles / l blocks)
NCH = L // 512      # 4 deg psum chunks of 512
DBN = D // 128      # 4 d-blocks
LN_EPS = 1e-5
DSCALE = float(D) ** -0.5
F32 = mybir.dt.float32
BF16 = mybir.dt.bfloat16
MUL = mybir.AluOpType.mult
ADD = mybir.AluOpType.add
SUB = mybir.AluOpType.subtract

LAST_RESULT = None  # BassKernelResults of the most recent run (for profiling)


def _build_program(fold_scale=True, ln_identity=True):
    """fold_scale: edge_weight folded into W host-side and bias == 0, so the
    relu collapses to an activation with per-partition scale dis_l*DSCALE.
    ln_identity: ln_w == 1, ln_b == 0."""
    nc = bacc.Bacc("TRN2", target_bir_lowering=False, debug=False)
    d = {}
    def di(name, shape, dt):
        d[name] = nc.dram_tensor(name, shape, dt, kind="ExternalInput").ap()
    di("ident", [128, 128], F32)
    di("aT_h", [L, L], BF16)
    di("xT_h", [D, L], BF16)
    di("x_in", [L, D], F32)
    di("wt_h", [D, D], BF16)
    if not fold_scale:
        di("ewc", [128, 1], F32)
        di("b_row", [1, D], F32)
    if not ln_identity:
        di("lnw_row", [1, D], F32)
        di("lnb_row", [1, D], F32)
    out_d = nc.dram_tensor("out_t", [L, D], F32, kind="ExternalOutput").ap()

    with tile.TileContext(nc) as tc:
        with tc.tile_pool(name="pA", bufs=JBN) as pA, \
             tc.tile_pool(name="pX", bufs=JBN) as pX, \
             tc.tile_pool(name="pY", bufs=JBN) as pY, \
             tc.tile_pool(name="pXW", bufs=JBN) as pXW, \
             tc.tile_pool(name="pW", bufs=DBN) as pW, \
             tc.tile_pool(name="pCol", bufs=12) as pCol, \
             tc.tile_pool(name="pSmall", bufs=1) as pSmall:

            # ---- persistent arrays ----
            aT_t = [pA.tile([128, L], BF16, tag="aT", name=f"aT{j}")
                    for j in range(JBN)]
            x_t = [pX.tile([128, D], F32, tag="x", name=f"x{j}") for j in range(JBN)]
            xw_t = [pXW.tile([128, D], BF16, tag="xw", name=f"xw{j}")
                    for j in range(JBN)]
            pXTh_cm = tc.tile_pool(name="pXTh", bufs=DBN)
            pXTh = pXTh_cm.__enter__()
            xTh_t = [pXTh.tile([128, L], BF16, tag="xTh", name=f"xTh{m}")
                     for m in range(DBN)]
            y_t = [pY.tile([128, D], BF16, tag="y", name=f"y{j}")
                   for j in range(JBN)]
            wt_t = [pW.tile([128, D], BF16, tag="wt", name=f"wt{k}")
                    for k in range(DBN)]
            eps_t = pSmall.tile([128, 1], F32, tag="eps")
            nc.vector.memset(eps_t[:], LN_EPS)
            onesc_t = pSmall.tile([128, 1], BF16, tag="onesc")
            nc.vector.memset(onesc_t[:], 1.0)
            # touch every activation function now so the Act table loads
            # happen during the DMA stream, not on the critical path later
            warm_t = pSmall.tile([128, 1], F32, tag="warm")
            nc.scalar.sqrt(warm_t[:], eps_t[:])
            nc.scalar.activation(warm_t[:], eps_t[:],
                                 mybir.ActivationFunctionType.Square)
            nc.scalar.activation(warm_t[:], eps_t[:],
                                 mybir.ActivationFunctionType.Relu)
            stat_b = {}

            # DMA order: ident, W, xT (for XW), aT (gates deg), x f32 last
            for k in range(DBN):
                nc.sync.dma_start(wt_t[k][:], d["wt_h"][k * 128:(k + 1) * 128, :])
                nc.sync.dma_start(xTh_t[k][:], d["xT_h"][k * 128:(k + 1) * 128, :])
            ident_t = pSmall.tile([128, 128], F32, tag="ident")
            nc.sync.dma_start(ident_t[:], d["ident"][:])
            for jb in range(JBN):
                nc.sync.dma_start(aT_t[jb][:], d["aT_h"][jb * 128:(jb + 1) * 128, :])
            for jb in range(JBN):
                nc.sync.dma_start(x_t[jb][:], d["x_in"][jb * 128:(jb + 1) * 128, :])
            if not fold_scale:
                ewc_t = pSmall.tile([128, 1], F32, tag="ew")
                nc.scalar.dma_start(ewc_t[:], d["ewc"][:])
                b_r = pSmall.tile([1, D], F32, tag="brow")
                nc.scalar.dma_start(b_r[:], d["b_row"][:])
                bbT = pSmall.tile([128, D], F32, tag="bb")
                nc.gpsimd.partition_broadcast(bbT[:], b_r[:])
            if not ln_identity:
                rows = {}
                for nm in ("lnw_row", "lnb_row"):
                    r = pSmall.tile([1, D], F32, tag=nm, name=nm + "_t")
                    nc.scalar.dma_start(r[:], d[nm][:])
                    rows[nm] = r
                for nm in ("lnw_row", "lnb_row"):
                    t = pSmall.tile([128, D], F32, tag=nm + "b", name=nm + "_b")
                    nc.gpsimd.partition_broadcast(t[:], rows[nm][:])
                    stat_b[nm] = t

            # ---- phase H: XW = x @ W.T on PE from the host-transposed x,
            # m-major over jb-halves in 8 PSUM banks, while aT streams ----
            with tc.tile_pool(name="psXW", bufs=8, space="PSUM") as psXW:
                for q in range(4):
                    jbs = range(q * 4, q * 4 + 4)
                    xwp = {jb: psXW.tile([128, D], F32, tag="xwp",
                                         name=f"xwp{jb}") for jb in jbs}
                    for m in range(DBN):
                        for jb in jbs:
                            nc.tensor.matmul(
                                xwp[jb][:],
                                xTh_t[m][:, jb * 128:(jb + 1) * 128],
                                wt_t[m][:],
                                start=(m == 0), stop=(m == DBN - 1))
                    for jb in jbs:
                        if jb % 2 == 0:
                            nc.vector.tensor_copy(xw_t[jb][:], xwp[jb][:])
                        else:
                            nc.scalar.copy(xw_t[jb][:], xwp[jb][:])
            pXTh_cm.__exit__(None, None, None)

            with tc.tile_pool(name="psDeg", bufs=2, space="PSUM") as psDeg, \
                 tc.tile_pool(name="psPT", bufs=1, space="PSUM") as psPT, \
                 tc.tile_pool(name="pTr", bufs=1) as pTr:
                # deg matvecs: 4 psum row accumulators over 16 aT tiles
                deg_ps = [psDeg.tile([128, 512], F32, tag="deg",
                                     name=f"deg_ps{i}") for i in range(2)]
                for jb in range(JBN):
                    for n in range(NCH):
                        po = 32 * (n % 2)
                        nc.tensor.matmul(
                            deg_ps[n // 2][po:po + 1, :],
                            onesc_t[:],
                            aT_t[jb][:, n * 512:(n + 1) * 512],
                            start=(jb == 0), stop=(jb == JBN - 1))

                # ---- phase T: deg -> dis -> y, two-stage pipeline ----
                r_sb = pTr.tile([128, 1024], F32, tag="rsb")
                rc_ps = psPT.tile([128, JBN], F32, tag="rc")
                std_col = pCol.tile([128, JBN], F32, tag="stdc", bufs=1)
                dis_col = pCol.tile([128, JBN], F32, tag="disc", bufs=1)
                for t in range(2):
                    csl = slice(t * 8, t * 8 + 8)
                    nc.vector.tensor_copy(r_sb[0:33, t * 512:(t + 1) * 512],
                                          deg_ps[t][0:33, :])
                    for v in range(t * 8, t * 8 + 8):
                        n, c = v // 4, v % 4
                        po = 32 * (n % 2)
                        fo = (n // 2) * 512 + c * 128
                        nc.tensor.transpose(
                            rc_ps[:, v:v + 1],
                            r_sb[po:po + 1, fo:fo + 128],
                            ident_t[po:po + 1, po:po + 1])
                    nc.scalar.sqrt(std_col[:, csl], rc_ps[:, csl])
                    nc.vector.reciprocal(dis_col[:, csl], std_col[:, csl])
                    for jb in range(t * 8, t * 8 + 8):
                        # bf16 in/out, SBUF only: rotate across DVE/Act/Pool
                        w = jb % 3
                        if w == 0:
                            nc.vector.tensor_scalar_mul(
                                y_t[jb][:], xw_t[jb][:], dis_col[:, jb:jb + 1])
                        elif w == 1:
                            nc.scalar.mul(y_t[jb][:], xw_t[jb][:],
                                          dis_col[:, jb:jb + 1])
                        else:
                            nc.gpsimd.tensor_scalar_mul(
                                y_t[jb][:], xw_t[jb][:], dis_col[:, jb:jb + 1])

            if fold_scale:
                c1s_col = pCol.tile([128, JBN], F32, tag="c1s", bufs=1)
                nc.scalar.mul(c1s_col[:], dis_col[:], DSCALE)
            else:
                c1_col = pCol.tile([128, JBN], F32, tag="c1c", bufs=1)
                nc.vector.tensor_scalar_mul(c1_col[:], dis_col[:], ewc_t[:])

            # ---- phase M: one matmul group + relu/LN chain per 128-row
            # block; 8 PSUM banks rotate, freed by the relu read ----
            with tc.tile_pool(name="psMM", bufs=8, space="PSUM") as psMM, \
                 tc.tile_pool(name="pScr", bufs=9) as pScr, \
                 tc.tile_pool(name="pOut", bufs=5) as pOut:
                for ib in range(JBN):
                    lb = ib
                    ps = psMM.tile([128, 512], F32, tag="mm", name=f"mm{ib}")
                    for jb in range(JBN):
                        nc.tensor.matmul(
                            ps[:], aT_t[jb][:, ib * 128:(ib + 1) * 128],
                            y_t[jb][:], start=(jb == 0), stop=(jb == JBN - 1))
                    r = pScr.tile([128, D], F32, tag="scr", name=f"r{lb}")
                    if fold_scale:
                        nc.scalar.activation(
                            r[:], ps[:], mybir.ActivationFunctionType.Relu,
                            scale=c1s_col[:, lb:lb + 1])
                    else:
                        tmp = pScr.tile([128, D], F32, tag="scr", name=f"tb{lb}")
                        nc.vector.scalar_tensor_tensor(
                            tmp[:], ps[:], c1_col[:, lb:lb + 1], bbT[:],
                            MUL, ADD)
                        nc.scalar.activation(
                            r[:], tmp[:], mybir.ActivationFunctionType.Relu,
                            scale=DSCALE)
                    hh = pScr.tile([128, D], F32, tag="scr", name=f"hh{lb}")
                    sums = pCol.tile([128, 1], F32, tag="lncol", name=f"su{lb}")
                    nc.vector.scalar_tensor_tensor(
                        hh[:], r[:], 1.0, x_t[lb][:], MUL, ADD,
                        accum_out=sums[:])
                    sq = pScr.tile([128, D], F32, tag="scr", name=f"sq{lb}")
                    m2s = pCol.tile([128, 1], F32, tag="lncol", name=f"m2{lb}")
                    nc.scalar.activation(
                        sq[:], hh[:], mybir.ActivationFunctionType.Square,
                        accum_out=m2s[:])
                    mu = pCol.tile([128, 1], F32, tag="lncol", name=f"mu{lb}")
                    nc.scalar.mul(mu[:], sums[:], 1.0 / D)
                    m2n = pCol.tile([128, 1], F32, tag="lncol", name=f"mn{lb}")
                    nc.scalar.mul(m2n[:], m2s[:], 1.0 / D)
                    negv = pCol.tile([128, 1], F32, tag="lncol", name=f"nv{lb}")
                    nc.vector.scalar_tensor_tensor(
                        negv[:], mu[:], mu[:], m2n[:], MUL, SUB)
                    stdt = pCol.tile([128, 1], F32, tag="lncol", name=f"sd{lb}")
                    nc.scalar.activation(
                        stdt[:], negv[:], mybir.ActivationFunctionType.Sqrt,
                        scale=-1.0, bias=eps_t[:])
                    rstd = pCol.tile([128, 1], F32, tag="lncol", name=f"rs{lb}")
                    nc.vector.reciprocal(rstd[:], stdt[:])
                    eng1 = nc.gpsimd if lb % 2 == 0 else nc.vector
                    t1 = pOut.tile([128, D], F32, tag="o", name=f"t1{lb}")
                    eng1.tensor_scalar(t1[:], hh[:], mu[:], rstd[:], SUB, MUL)
                    if ln_identity:
                        nc.scalar.dma_start(
                            out_d[lb * 128:(lb + 1) * 128, :], t1[:])
                    else:
                        tt = pScr.tile([128, D], F32, tag="scr", name=f"tt{lb}")
                        teng = nc.vector if lb % 2 == 0 else nc.gpsimd
                        teng.tensor_mul(tt[:], t1[:], stat_b["lnw_row"][:])
                        o_sb = pOut.tile([128, D], F32, tag="o", name=f"o{lb}")
                        nc.gpsimd.tensor_add(o_sb[:], tt[:],
                                             stat_b["lnb_row"][:])
                        nc.scalar.dma_start(
                            out_d[lb * 128:(lb + 1) * 128, :], o_sb[:])

    nc.compile()
    return nc


_NC_CACHE = {}


def _get_nc(fold_scale=True, ln_identity=True):
    key = (fold_scale, ln_identity)
    if key not in _NC_CACHE:
        _NC_CACHE[key] = _build_program(*key)
    return _NC_CACHE[key]


def kernel(x, adj, pad_mask, W, b, ln_w, ln_b, edge_weight):
    global LAST_RESULT
    x = np.asarray(x, dtype=np.float32)
    adj = np.asarray(adj, dtype=np.float32)
    pad_mask = np.asarray(pad_mask)
    W = np.asarray(W, dtype=np.float32)
    b = np.asarray(b, dtype=np.float32)
    ln_w = np.asarray(ln_w, dtype=np.float32)
    ln_b = np.asarray(ln_b, dtype=np.float32)
    ew = float(np.asarray(edge_weight).reshape(-1)[0])

    ln_identity = bool(np.all(ln_w == 1.0) and np.all(ln_b == 0.0))
    fold_scale = bool(np.all(b == 0.0) and ew >= 0.0)
    nc = _get_nc(fold_scale, ln_identity)

    ident = np.eye(128, dtype=np.float32)
    # fold_scale: ew commutes with W inside the relu argument, so fold it
    # into the weights host-side; deg/dis stay ew-free.
    w_eff = W.T * ew if fold_scale else W.T
    wt_h = np.ascontiguousarray(w_eff).astype(ml_dtypes.bfloat16)
    eye = np.eye(L, dtype=np.float32)

    in_maps = []
    for c in range(B):
        valid = (~pad_mask[c]).astype(np.float32)
        aT = (adj[c].T * valid[:, None]) * valid[None, :]
        aT += eye
        im = {
            "ident": ident,
            "aT_h": aT.astype(ml_dtypes.bfloat16),
            "xT_h": np.ascontiguousarray(x[c].T).astype(ml_dtypes.bfloat16),
            "x_in": np.ascontiguousarray(x[c]),
            "wt_h": wt_h,
        }
        if not fold_scale:
            im["ewc"] = np.full((128, 1), ew, dtype=np.float32)
            im["b_row"] = np.ascontiguousarray(b.reshape(1, D))
        if not ln_identity:
            im["lnw_row"] = np.ascontiguousarray(ln_w.reshape(1, D))
            im["lnb_row"] = np.ascontiguousarray(ln_b.reshape(1, D))
        in_maps.append(im)

    trace = os.environ.get("KERNEL_TRACE", "0") == "1"
    res = run_bass_kernel_spmd(nc, in_maps, core_ids=list(range(B)), trace=trace)
    LAST_RESULT = res
    out = np.stack([res.results[c]["out_t"] for c in range(B)], axis=0)
    return out


# revision 17
# speedup vs baseline: 1.4401x; 1.0037x over previous
"""GCN layer kernel for TRN2, data-parallel over batch across 8 NeuronCores.

v4: the two matmuls compose linearly (relu sits after BOTH), so
  out2[l,o] = dis_l * sum_j aT[j,l] * (dis_j * (x @ W.T)[j,o])
XW = x @ W.T is computed on the PE during the otherwise idle adjacency
stream window (x tiles are DMA'd first), via PE transposes of x. After deg
-> dis, y = dis * XW (bf16) becomes the moving operand of the single big
matmul with stationary aT column slices, which lands the result directly in
[l, o] layout, one PSUM bank per 128-row block: relu (with the per-partition
dis_l*DSCALE scale) reads PSUM straight, then residual + layernorm. No
second matmul pass, no PSUM->SBUF agg drains, no transposed output fixups.

Phases:
  H: DMA ident, x0, W, x1.., aT0..15. PE: per x tile, 4 transposes ->
     xT bf16, 4 matmuls -> XW psum -> bf16 SBUF; then deg matvecs ride
     the aT stream (PSUM accum over 16 j tiles).
  T: deg rows -> col layout (PE transposes), sqrt off PSUM, reciprocal;
     y[jb] = xw[jb] * dis_col[jb] on DVE/Pool alternating.
  M: 16 i-blocks x (16 j accumulation matmuls), relu+LN chain per block,
     stream out.
"""
import os
import numpy as np
import ml_dtypes

import concourse.bacc as bacc
import concourse.tile as tile
import concourse.mybir as mybir
from concourse.bass_utils import run_bass_kernel_spmd

B, L, D = 8, 2048, 512
JBN = L // 128      # 16 row blocks (j tiles / l blocks)
NCH = L // 512      # 4 deg psum chunks of 512
DBN = D // 128      # 4 d-blocks
LN_EPS = 1e-5
DSCALE = float(D) ** -0.5
F32 = mybir.dt.float32
BF16 = mybir.dt.bfloat16
MUL = mybir.AluOpType.mult
ADD = mybir.AluOpType.add
SUB = mybir.AluOpType.subtract

LAST_RESULT = None  # BassKernelResults of the most recent run (for profiling)


def _build_program(fold_scale=True, ln_identity=True):
    """fold_scale: edge_weight folded into W host-side and bias == 0, so the
    relu collapses to an activation with per-partition scale dis_l*DSCALE.
    ln_identity: ln_w == 1, ln_b == 0."""
    nc = bacc.Bacc("TRN2", target_bir_lowering=False, debug=False)
    d = {}
    def di(name, shape, dt):
        d[name] = nc.dram_tensor(name, shape, dt, kind="ExternalInput").ap()
    di("ident", [128, 128], F32)
    di("aT_h", [L, L], BF16)
    di("xT_h", [D, L], BF16)
    di("x_in", [L, D], F32)
    di("wt_h", [D, D], BF16)
    if not fold_scale:
        di("ewc", [128, 1], F32)
        di("b_row", [1, D], F32)
    if not ln_identity:
        di("lnw_row", [1, D], F32)
        di("lnb_row", [1, D], F32)
    out_d = nc.dram_tensor("out_t", [L, D], F32, kind="ExternalOutput").ap()

    with tile.TileContext(nc) as tc:
        with tc.tile_pool(name="pA", bufs=JBN) as pA, \
             tc.tile_pool(name="pX", bufs=JBN) as pX, \
             tc.tile_pool(name="pY", bufs=JBN) as pY, \
             tc.tile_pool(name="pXW", bufs=JBN) as pXW, \
             tc.tile_pool(name="pW", bufs=DBN) as pW, \
             tc.tile_pool(name="pCol", bufs=12) as pCol, \
             tc.tile_pool(name="pSmall", bufs=1) as pSmall:

            # ---- persistent arrays ----
            aT_t = [pA.tile([128, L], BF16, tag="aT", name=f"aT{j}")
                    for j in range(JBN)]
            x_t = [pX.tile([128, D], F32, tag="x", name=f"x{j}") for j in range(JBN)]
            xw_t = [pXW.tile([128, D], BF16, tag="xw", name=f"xw{j}")
                    for j in range(JBN)]
            pXTh_cm = tc.tile_pool(name="pXTh", bufs=DBN)
            pXTh = pXTh_cm.__enter__()
            xTh_t = [pXTh.tile([128, L], BF16, tag="xTh", name=f"xTh{m}")
                     for m in range(DBN)]
            y_t = [pY.tile([128, D], BF16, tag="y", name=f"y{j}")
                   for j in range(JBN)]
            wt_t = [pW.tile([128, D], BF16, tag="wt", name=f"wt{k}")
                    for k in range(DBN)]
            eps_t = pSmall.tile([128, 1], F32, tag="eps")
            nc.vector.memset(eps_t[:], LN_EPS)
            onesc_t = pSmall.tile([128, 1], BF16, tag="onesc")
            nc.vector.memset(onesc_t[:], 1.0)
            # touch every activation function now so the Act table loads
            # happen during the DMA stream, not on the critical path later
            warm_t = pSmall.tile([128, 1], F32, tag="warm")
            nc.scalar.sqrt(warm_t[:], eps_t[:])
            nc.scalar.activation(warm_t[:], eps_t[:],
                                 mybir.ActivationFunctionType.Square)
            nc.scalar.activation(warm_t[:], eps_t[:],
                                 mybir.ActivationFunctionType.Relu)
            stat_b = {}

            # DMA order: ident, W, xT (for XW), aT (gates deg), x f32 last
            for k in range(DBN):
                nc.sync.dma_start(wt_t[k][:], d["wt_h"][k * 128:(k + 1) * 128, :])
                nc.sync.dma_start(xTh_t[k][:], d["xT_h"][k * 128:(k + 1) * 128, :])
            ident_t = pSmall.tile([128, 128], F32, tag="ident")
            nc.sync.dma_start(ident_t[:], d["ident"][:])
            for jb in range(JBN):
                nc.sync.dma_start(aT_t[jb][:], d["aT_h"][jb * 128:(jb + 1) * 128, :])
            for jb in range(JBN):
                nc.sync.dma_start(x_t[jb][:], d["x_in"][jb * 128:(jb + 1) * 128, :])
            if not fold_scale:
                ewc_t = pSmall.tile([128, 1], F32, tag="ew")
                nc.scalar.dma_start(ewc_t[:], d["ewc"][:])
                b_r = pSmall.tile([1, D], F32, tag="brow")
                nc.scalar.dma_start(b_r[:], d["b_row"][:])
                bbT = pSmall.tile([128, D], F32, tag="bb")
                nc.gpsimd.partition_broadcast(bbT[:], b_r[:])
            if not ln_identity:
                rows = {}
                for nm in ("lnw_row", "lnb_row"):
                    r = pSmall.tile([1, D], F32, tag=nm, name=nm + "_t")
                    nc.scalar.dma_start(r[:], d[nm][:])
                    rows[nm] = r
                for nm in ("lnw_row", "lnb_row"):
                    t = pSmall.tile([128, D], F32, tag=nm + "b", name=nm + "_b")
                    nc.gpsimd.partition_broadcast(t[:], rows[nm][:])
                    stat_b[nm] = t

            # ---- phase H: XW = x @ W.T on PE from the host-transposed x,
            # m-major over jb-halves in 8 PSUM banks, while aT streams ----
            with tc.tile_pool(name="psXW", bufs=8, space="PSUM") as psXW:
                for q in range(4):
                    jbs = range(q * 4, q * 4 + 4)
                    xwp = {jb: psXW.tile([128, D], F32, tag="xwp",
                                         name=f"xwp{jb}") for jb in jbs}
                    for m in range(DBN):
                        for jb in jbs:
                            nc.tensor.matmul(
                                xwp[jb][:],
                                xTh_t[m][:, jb * 128:(jb + 1) * 128],
                                wt_t[m][:],
                                start=(m == 0), stop=(m == DBN - 1))
                    for jb in jbs:
                        if jb % 2 == 0:
                            nc.vector.tensor_copy(xw_t[jb][:], xwp[jb][:])
                        else:
                            nc.scalar.copy(xw_t[jb][:], xwp[jb][:])
            pXTh_cm.__exit__(None, None, None)

            with tc.tile_pool(name="psDeg", bufs=2, space="PSUM") as psDeg, \
                 tc.tile_pool(name="psPT", bufs=1, space="PSUM") as psPT, \
                 tc.tile_pool(name="pTr", bufs=1) as pTr:
                # deg matvecs: 4 psum row accumulators over 16 aT tiles
                deg_ps = [psDeg.tile([128, 512], F32, tag="deg",
                                     name=f"deg_ps{i}") for i in range(2)]
                for jb in range(JBN):
                    for n in range(NCH):
                        po = 32 * (n % 2)
                        nc.tensor.matmul(
                            deg_ps[n // 2][po:po + 1, :],
                            onesc_t[:],
                            aT_t[jb][:, n * 512:(n + 1) * 512],
                            start=(jb == 0), stop=(jb == JBN - 1))

                # ---- phase T: deg -> dis -> y, two-stage pipeline ----
                r_sb = pTr.tile([128, 1024], F32, tag="rsb")
                rc_ps = psPT.tile([128, JBN], F32, tag="rc")
                std_col = pCol.tile([128, JBN], F32, tag="stdc", bufs=1)
                dis_col = pCol.tile([128, JBN], F32, tag="disc", bufs=1)
                for t in range(2):
                    csl = slice(t * 8, t * 8 + 8)
                    nc.vector.tensor_copy(r_sb[0:33, t * 512:(t + 1) * 512],
                                          deg_ps[t][0:33, :])
                    for v in range(t * 8, t * 8 + 8):
                        n, c = v // 4, v % 4
                        po = 32 * (n % 2)
                        fo = (n // 2) * 512 + c * 128
                        nc.tensor.transpose(
                            rc_ps[:, v:v + 1],
                            r_sb[po:po + 1, fo:fo + 128],
                            ident_t[po:po + 1, po:po + 1])
                    nc.scalar.sqrt(std_col[:, csl], rc_ps[:, csl])
                    nc.vector.reciprocal(dis_col[:, csl], std_col[:, csl])
                    for jb in range(t * 8, t * 8 + 8):
                        # bf16 in/out, SBUF only: rotate across DVE/Act/Pool
                        w = jb % 3
                        if w == 0:
                            nc.vector.tensor_scalar_mul(
                                y_t[jb][:], xw_t[jb][:], dis_col[:, jb:jb + 1])
                        elif w == 1:
                            nc.scalar.mul(y_t[jb][:], xw_t[jb][:],
                                          dis_col[:, jb:jb + 1])
                        else:
                            nc.gpsimd.tensor_scalar_mul(
                                y_t[jb][:], xw_t[jb][:], dis_col[:, jb:jb + 1])

            if fold_scale:
                c1s_col = pCol.tile([128, JBN], F32, tag="c1s", bufs=1)
                nc.scalar.mul(c1s_col[:], dis_col[:], DSCALE)
            else:
                c1_col = pCol.tile([128, JBN], F32, tag="c1c", bufs=1)
                nc.vector.tensor_scalar_mul(c1_col[:], dis_col[:], ewc_t[:])

            # ---- phase M: one matmul group + relu/LN chain per 128-row
            # block; 8 PSUM banks rotate, freed by the relu read ----
            with tc.tile_pool(name="psMM", bufs=8, space="PSUM") as psMM, \
                 tc.tile_pool(name="pScr", bufs=9) as pScr, \
                 tc.tile_pool(name="pOut", bufs=5) as pOut:
                for ib in range(JBN):
                    lb = ib
                    ps = psMM.tile([128, 512], F32, tag="mm", name=f"mm{ib}")
                    for jb in range(JBN):
                        nc.tensor.matmul(
                            ps[:], aT_t[jb][:, ib * 128:(ib + 1) * 128],
                            y_t[jb][:], start=(jb == 0), stop=(jb == JBN - 1))
                    r = pScr.tile([128, D], F32, tag="scr", name=f"r{lb}")
                    if fold_scale:
                        nc.scalar.activation(
                            r[:], ps[:], mybir.ActivationFunctionType.Relu,
                            scale=c1s_col[:, lb:lb + 1])
                    else:
                        tmp = pScr.tile([128, D], F32, tag="scr", name=f"tb{lb}")
                        nc.vector.scalar_tensor_tensor(
                            tmp[:], ps[:], c1_col[:, lb:lb + 1], bbT[:],
                            MUL, ADD)
                        nc.scalar.activation(
                            r[:], tmp[:], mybir.ActivationFunctionType.Relu,
                            scale=DSCALE)
                    hh = pScr.tile([128, D], F32, tag="scr", name=f"hh{lb}")
                    nc.vector.scalar_tensor_tensor(
                        hh[:], r[:], 1.0, x_t[lb][:], MUL, ADD)
                    st6 = pCol.tile([128, 6], F32, tag="lnst", name=f"st{lb}")
                    nc.vector.bn_stats(st6[:], hh[:])
                    mv = pCol.tile([128, 2], F32, tag="lnmv", name=f"mv{lb}")
                    nc.vector.bn_aggr(mv[:], st6[:])
                    stdt = pCol.tile([128, 1], F32, tag="lncol", name=f"sd{lb}")
                    nc.scalar.activation(
                        stdt[:], mv[:, 1:2], mybir.ActivationFunctionType.Sqrt,
                        bias=eps_t[:])
                    rstd = pCol.tile([128, 1], F32, tag="lncol", name=f"rs{lb}")
                    nc.vector.reciprocal(rstd[:], stdt[:])
                    eng1 = nc.gpsimd if lb % 2 == 0 else nc.vector
                    t1 = pOut.tile([128, D], F32, tag="o", name=f"t1{lb}")
                    eng1.tensor_scalar(t1[:], hh[:], mv[:, 0:1], rstd[:],
                                       SUB, MUL)
                    if ln_identity:
                        nc.scalar.dma_start(
                            out_d[lb * 128:(lb + 1) * 128, :], t1[:])
                    else:
                        tt = pScr.tile([128, D], F32, tag="scr", name=f"tt{lb}")
                        teng = nc.vector if lb % 2 == 0 else nc.gpsimd
                        teng.tensor_mul(tt[:], t1[:], stat_b["lnw_row"][:])
                        o_sb = pOut.tile([128, D], F32, tag="o", name=f"o{lb}")
                        nc.gpsimd.tensor_add(o_sb[:], tt[:],
                                             stat_b["lnb_row"][:])
                        nc.scalar.dma_start(
                            out_d[lb * 128:(lb + 1) * 128, :], o_sb[:])

    nc.compile()
    return nc


_NC_CACHE = {}


def _get_nc(fold_scale=True, ln_identity=True):
    key = (fold_scale, ln_identity)
    if key not in _NC_CACHE:
        _NC_CACHE[key] = _build_program(*key)
    return _NC_CACHE[key]


def kernel(x, adj, pad_mask, W, b, ln_w, ln_b, edge_weight):
    global LAST_RESULT
    x = np.asarray(x, dtype=np.float32)
    adj = np.asarray(adj, dtype=np.float32)
    pad_mask = np.asarray(pad_mask)
    W = np.asarray(W, dtype=np.float32)
    b = np.asarray(b, dtype=np.float32)
    ln_w = np.asarray(ln_w, dtype=np.float32)
    ln_b = np.asarray(ln_b, dtype=np.float32)
    ew = float(np.asarray(edge_weight).reshape(-1)[0])

    ln_identity = bool(np.all(ln_w == 1.0) and np.all(ln_b == 0.0))
    fold_scale = bool(np.all(b == 0.0) and ew >= 0.0)
    nc = _get_nc(fold_scale, ln_identity)

    ident = np.eye(128, dtype=np.float32)
    # fold_scale: ew commutes with W inside the relu argument, so fold it
    # into the weights host-side; deg/dis stay ew-free.
    w_eff = W.T * ew if fold_scale else W.T
    wt_h = np.ascontiguousarray(w_eff).astype(ml_dtypes.bfloat16)
    eye = np.eye(L, dtype=np.float32)

    in_maps = []
    for c in range(B):
        valid = (~pad_mask[c]).astype(np.float32)
        aT = (adj[c].T * valid[:, None]) * valid[None, :]
        aT += eye
        im = {
            "ident": ident,
            "aT_h": aT.astype(ml_dtypes.bfloat16),
            "xT_h": np.ascontiguousarray(x[c].T).astype(ml_dtypes.bfloat16),
            "x_in": np.ascontiguousarray(x[c]),
            "wt_h": wt_h,
        }
        if not fold_scale:
            im["ewc"] = np.full((128, 1), ew, dtype=np.float32)
            im["b_row"] = np.ascontiguousarray(b.reshape(1, D))
        if not ln_identity:
            im["lnw_row"] = np.ascontiguousarray(ln_w.reshape(1, D))
            im["lnb_row"] = np.ascontiguousarray(ln_b.reshape(1, D))
        in_maps.append(im)

    trace = os.environ.get("KERNEL_TRACE", "0") == "1"
    res = run_bass_kernel_spmd(nc, in_maps, core_ids=list(range(B)), trace=trace)
    LAST_RESULT = res
    out = np.stack([res.results[c]["out_t"] for c in range(B)], axis=0)
    return out


# revision 19
# speedup vs baseline: 1.5036x; 1.0441x over previous
"""GCN layer kernel for TRN2, data-parallel over batch across 8 NeuronCores.

v4: the two matmuls compose linearly (relu sits after BOTH), so
  out2[l,o] = dis_l * sum_j aT[j,l] * (dis_j * (x @ W.T)[j,o])
XW = x @ W.T is computed on the PE during the otherwise idle adjacency
stream window (x tiles are DMA'd first), via PE transposes of x. After deg
-> dis, y = dis * XW (bf16) becomes the moving operand of the single big
matmul with stationary aT column slices, which lands the result directly in
[l, o] layout, one PSUM bank per 128-row block: relu (with the per-partition
dis_l*DSCALE scale) reads PSUM straight, then residual + layernorm. No
second matmul pass, no PSUM->SBUF agg drains, no transposed output fixups.

Phases:
  H: DMA ident, x0, W, x1.., aT0..15. PE: per x tile, 4 transposes ->
     xT bf16, 4 matmuls -> XW psum -> bf16 SBUF; then deg matvecs ride
     the aT stream (PSUM accum over 16 j tiles).
  T: deg rows -> col layout (PE transposes), sqrt off PSUM, reciprocal;
     y[jb] = xw[jb] * dis_col[jb] on DVE/Pool alternating.
  M: 16 i-blocks x (16 j accumulation matmuls), relu+LN chain per block,
     stream out.
"""
import os
import numpy as np
import ml_dtypes

import concourse.bacc as bacc
import concourse.tile as tile
import concourse.mybir as mybir
from concourse.bass_utils import run_bass_kernel_spmd

B, L, D = 8, 2048, 512
JBN = L // 128      # 16 row blocks (j tiles / l blocks)
NCH = L // 512      # 4 deg psum chunks of 512
DBN = D // 128      # 4 d-blocks
LN_EPS = 1e-5
DSCALE = float(D) ** -0.5
F32 = mybir.dt.float32
BF16 = mybir.dt.bfloat16
MUL = mybir.AluOpType.mult
ADD = mybir.AluOpType.add
SUB = mybir.AluOpType.subtract

LAST_RESULT = None  # BassKernelResults of the most recent run (for profiling)


def _build_program(fold_scale=True, ln_identity=True):
    """fold_scale: edge_weight folded into W host-side and bias == 0, so the
    relu collapses to an activation with per-partition scale dis_l*DSCALE.
    ln_identity: ln_w == 1, ln_b == 0."""
    nc = bacc.Bacc("TRN2", target_bir_lowering=False, debug=False)
    d = {}
    def di(name, shape, dt):
        d[name] = nc.dram_tensor(name, shape, dt, kind="ExternalInput").ap()
    di("ident", [128, 128], F32)
    di("aT_h", [L, L], BF16)
    di("xT_h", [D, L], BF16)
    di("x_in", [L, D], F32)
    di("wt_h", [D, D], BF16)
    if not fold_scale:
        di("ewc", [128, 1], F32)
        di("b_row", [1, D], F32)
    if not ln_identity:
        di("lnw_row", [1, D], F32)
        di("lnb_row", [1, D], F32)
    out_d = nc.dram_tensor("out_t", [L, D], F32, kind="ExternalOutput").ap()

    with tile.TileContext(nc) as tc:
        with tc.tile_pool(name="pA", bufs=JBN) as pA, \
             tc.tile_pool(name="pX", bufs=JBN) as pX, \
             tc.tile_pool(name="pY", bufs=JBN) as pY, \
             tc.tile_pool(name="pXW", bufs=JBN) as pXW, \
             tc.tile_pool(name="pW", bufs=DBN) as pW, \
             tc.tile_pool(name="pCol", bufs=12) as pCol, \
             tc.tile_pool(name="pSmall", bufs=1) as pSmall:

            # ---- persistent arrays ----
            aT_t = [pA.tile([128, L], BF16, tag="aT", name=f"aT{j}")
                    for j in range(JBN)]
            x_t = [pX.tile([128, D], F32, tag="x", name=f"x{j}") for j in range(JBN)]
            xw_t = [pXW.tile([128, D], BF16, tag="xw", name=f"xw{j}")
                    for j in range(JBN)]
            pXTh_cm = tc.tile_pool(name="pXTh", bufs=DBN)
            pXTh = pXTh_cm.__enter__()
            xTh_t = [pXTh.tile([128, L], BF16, tag="xTh", name=f"xTh{m}")
                     for m in range(DBN)]
            y_t = [pY.tile([128, D], BF16, tag="y", name=f"y{j}")
                   for j in range(JBN)]
            wt_t = [pW.tile([128, D], BF16, tag="wt", name=f"wt{k}")
                    for k in range(DBN)]
            eps_t = pSmall.tile([128, 1], F32, tag="eps")
            nc.vector.memset(eps_t[:], LN_EPS)
            onesc_t = pSmall.tile([128, 1], BF16, tag="onesc")
            nc.vector.memset(onesc_t[:], 1.0)
            # touch every activation function now so the Act table loads
            # happen during the DMA stream, not on the critical path later
            warm_t = pSmall.tile([128, 1], F32, tag="warm")
            nc.scalar.sqrt(warm_t[:], eps_t[:])
            nc.scalar.activation(warm_t[:], eps_t[:],
                                 mybir.ActivationFunctionType.Square)
            nc.scalar.activation(warm_t[:], eps_t[:],
                                 mybir.ActivationFunctionType.Relu)
            stat_b = {}

            # DMA order: ident, W, xT (for XW), aT (gates deg), x f32 last
            for k in range(DBN):
                nc.sync.dma_start(wt_t[k][:], d["wt_h"][k * 128:(k + 1) * 128, :])
                rsl = slice(k * 128, (k + 1) * 128)
                if k < 2:
                    # split: the first 512 columns feed XW quarter 0 sooner
                    nc.sync.dma_start(xTh_t[k][:, 0:512], d["xT_h"][rsl, 0:512])
                    nc.sync.dma_start(xTh_t[k][:, 512:L], d["xT_h"][rsl, 512:L])
                else:
                    nc.sync.dma_start(xTh_t[k][:], d["xT_h"][rsl, :])
            ident_t = pSmall.tile([128, 128], F32, tag="ident")
            nc.sync.dma_start(ident_t[:], d["ident"][:])
            for jb in range(JBN):
                nc.sync.dma_start(aT_t[jb][:], d["aT_h"][jb * 128:(jb + 1) * 128, :])
            for jb in range(JBN):
                nc.sync.dma_start(x_t[jb][:], d["x_in"][jb * 128:(jb + 1) * 128, :])
            if not fold_scale:
                ewc_t = pSmall.tile([128, 1], F32, tag="ew")
                nc.scalar.dma_start(ewc_t[:], d["ewc"][:])
                b_r = pSmall.tile([1, D], F32, tag="brow")
                nc.scalar.dma_start(b_r[:], d["b_row"][:])
                bbT = pSmall.tile([128, D], F32, tag="bb")
                nc.gpsimd.partition_broadcast(bbT[:], b_r[:])
            if not ln_identity:
                rows = {}
                for nm in ("lnw_row", "lnb_row"):
                    r = pSmall.tile([1, D], F32, tag=nm, name=nm + "_t")
                    nc.scalar.dma_start(r[:], d[nm][:])
                    rows[nm] = r
                for nm in ("lnw_row", "lnb_row"):
                    t = pSmall.tile([128, D], F32, tag=nm + "b", name=nm + "_b")
                    nc.gpsimd.partition_broadcast(t[:], rows[nm][:])
                    stat_b[nm] = t

            # ---- phase H: XW = x @ W.T on PE from the host-transposed x,
            # m-major over jb-halves in 8 PSUM banks, while aT streams ----
            # XW emission is interleaved with the deg matvecs: quarter 0
            # runs before the aT stream lands; the remaining 48 XW matmuls
            # are doled out 3 per deg tile so the PE rides the stream with
            # no idle (per-tile slot: 4 deg matvecs + 3 XW matmuls ~= the
            # 1.46us tile DMA cadence).
            psXW_cm = tc.tile_pool(name="psXW", bufs=5, space="PSUM")
            psXW = psXW_cm.__enter__()

            def xw_instruction_stream():
                for q in range(4):
                    jbs = range(q * 4, q * 4 + 4)
                    xwp = {jb: psXW.tile([128, D], F32, tag="xwp",
                                         name=f"xwp{jb}") for jb in jbs}
                    for m in range(DBN):
                        for jb in jbs:
                            yield "mm", (lambda jb=jb, m=m, xwp=xwp:
                                nc.tensor.matmul(
                                    xwp[jb][:],
                                    xTh_t[m][:, jb * 128:(jb + 1) * 128],
                                    wt_t[m][:],
                                    start=(m == 0), stop=(m == DBN - 1)))
                    for jb in jbs:
                        if jb % 2 == 0:
                            yield "drain", (lambda jb=jb, xwp=xwp:
                                nc.vector.tensor_copy(xw_t[jb][:], xwp[jb][:]))
                        else:
                            yield "drain", (lambda jb=jb, xwp=xwp:
                                nc.scalar.copy(xw_t[jb][:], xwp[jb][:]))

            xw_stream = xw_instruction_stream()
            def emit_xw(k):
                n = 0
                for kind, op in xw_stream:
                    op()
                    if kind == "mm":
                        n += 1
                        if n >= k:
                            break

            emit_xw(20)  # quarter 0 + the head of quarter 1

            with tc.tile_pool(name="psDeg", bufs=2, space="PSUM") as psDeg, \
                 tc.tile_pool(name="psPT", bufs=1, space="PSUM") as psPT, \
                 tc.tile_pool(name="pTr", bufs=1) as pTr:
                # deg matvecs: 4 psum row accumulators over 16 aT tiles
                deg_ps = [psDeg.tile([128, 512], F32, tag="deg",
                                     name=f"deg_ps{i}") for i in range(2)]
                for jb in range(JBN):
                    for n in range(NCH):
                        po = 32 * (n % 2)
                        nc.tensor.matmul(
                            deg_ps[n // 2][po:po + 1, :],
                            onesc_t[:],
                            aT_t[jb][:, n * 512:(n + 1) * 512],
                            start=(jb == 0), stop=(jb == JBN - 1))
                    emit_xw(3)
                emit_xw(1000)  # drain any remaining XW work
                # ---- phase T: deg -> dis -> y, two-stage pipeline ----
                r_sb = pTr.tile([128, 1024], F32, tag="rsb")
                rc_ps = psPT.tile([128, JBN], F32, tag="rc")
                std_col = pCol.tile([128, JBN], F32, tag="stdc", bufs=1)
                dis_col = pCol.tile([128, JBN], F32, tag="disc", bufs=1)
                for t in range(2):
                    csl = slice(t * 8, t * 8 + 8)
                    if t == 0:
                        nc.vector.tensor_copy(r_sb[0:33, 0:512],
                                              deg_ps[0][0:33, :])
                    else:
                        nc.scalar.copy(r_sb[0:33, 512:1024],
                                       deg_ps[1][0:33, :])
                    for v in range(t * 8, t * 8 + 8):
                        n, c = v // 4, v % 4
                        po = 32 * (n % 2)
                        fo = (n // 2) * 512 + c * 128
                        nc.tensor.transpose(
                            rc_ps[:, v:v + 1],
                            r_sb[po:po + 1, fo:fo + 128],
                            ident_t[po:po + 1, po:po + 1])
                    nc.scalar.sqrt(std_col[:, csl], rc_ps[:, csl])
                    nc.vector.reciprocal(dis_col[:, csl], std_col[:, csl])
                    for jb in range(t * 8, t * 8 + 8):
                        # bf16 in/out, SBUF only: rotate across DVE/Act/Pool
                        w = jb % 3
                        if w == 0:
                            nc.vector.tensor_scalar_mul(
                                y_t[jb][:], xw_t[jb][:], dis_col[:, jb:jb + 1])
                        elif w == 1:
                            nc.scalar.mul(y_t[jb][:], xw_t[jb][:],
                                          dis_col[:, jb:jb + 1])
                        else:
                            nc.gpsimd.tensor_scalar_mul(
                                y_t[jb][:], xw_t[jb][:], dis_col[:, jb:jb + 1])

            psXW_cm.__exit__(None, None, None)
            pXTh_cm.__exit__(None, None, None)

            if fold_scale:
                c1s_col = pCol.tile([128, JBN], F32, tag="c1s", bufs=1)
                nc.scalar.mul(c1s_col[:], dis_col[:], DSCALE)
            else:
                c1_col = pCol.tile([128, JBN], F32, tag="c1c", bufs=1)
                nc.vector.tensor_scalar_mul(c1_col[:], dis_col[:], ewc_t[:])

            # ---- phase M: one matmul group + relu/LN chain per 128-row
            # block; 8 PSUM banks rotate, freed by the relu read ----
            with tc.tile_pool(name="psMM", bufs=8, space="PSUM") as psMM, \
                 tc.tile_pool(name="pScr", bufs=9) as pScr, \
                 tc.tile_pool(name="pOut", bufs=5) as pOut:
                for ib in range(JBN):
                    lb = ib
                    ps = psMM.tile([128, 512], F32, tag="mm", name=f"mm{ib}")
                    # last block: split into o-halves so half-a's relu/hh/
                    # stats overlap half-b's matmuls, shortening the tail
                    halves = ((slice(0, 512),),) if ib != JBN - 1 else \
                        ((slice(0, 256),), (slice(256, 512),))
                    r = pScr.tile([128, D], F32, tag="scr", name=f"r{lb}")
                    hh = pScr.tile([128, D], F32, tag="scr", name=f"hh{lb}")
                    st6 = pCol.tile([128, len(halves), 6], F32, tag="lnst",
                                    name=f"st{lb}")
                    for hi, (osl,) in enumerate(halves):
                        for jb in range(JBN):
                            nc.tensor.matmul(
                                ps[:, osl], aT_t[jb][:, ib * 128:(ib + 1) * 128],
                                y_t[jb][:, osl], start=(jb == 0),
                                stop=(jb == JBN - 1))
                        if fold_scale:
                            nc.scalar.activation(
                                r[:, osl], ps[:, osl],
                                mybir.ActivationFunctionType.Relu,
                                scale=c1s_col[:, lb:lb + 1])
                        else:
                            tmp = pScr.tile([128, D], F32, tag="scr",
                                            name=f"tb{lb}_{hi}")
                            nc.vector.scalar_tensor_tensor(
                                tmp[:, osl], ps[:, osl], c1_col[:, lb:lb + 1],
                                bbT[:, osl], MUL, ADD)
                            nc.scalar.activation(
                                r[:, osl], tmp[:, osl],
                                mybir.ActivationFunctionType.Relu,
                                scale=DSCALE)
                        nc.vector.scalar_tensor_tensor(
                            hh[:, osl], r[:, osl], 1.0, x_t[lb][:, osl],
                            MUL, ADD)
                        nc.vector.bn_stats(st6[:, hi, :], hh[:, osl])
                    mv = pCol.tile([128, 2], F32, tag="lnmv", name=f"mv{lb}")
                    nc.vector.bn_aggr(mv[:], st6[:])
                    stdt = pCol.tile([128, 1], F32, tag="lncol", name=f"sd{lb}")
                    nc.scalar.activation(
                        stdt[:], mv[:, 1:2], mybir.ActivationFunctionType.Sqrt,
                        bias=eps_t[:])
                    rstd = pCol.tile([128, 1], F32, tag="lncol", name=f"rs{lb}")
                    nc.vector.reciprocal(rstd[:], stdt[:])
                    eng1 = nc.gpsimd if lb % 2 == 0 else nc.vector
                    t1 = pOut.tile([128, D], F32, tag="o", name=f"t1{lb}")
                    eng1.tensor_scalar(t1[:], hh[:], mv[:, 0:1], rstd[:],
                                       SUB, MUL)
                    if ln_identity:
                        nc.scalar.dma_start(
                            out_d[lb * 128:(lb + 1) * 128, :], t1[:])
                    else:
                        tt = pScr.tile([128, D], F32, tag="scr", name=f"tt{lb}")
                        teng = nc.vector if lb % 2 == 0 else nc.gpsimd
                        teng.tensor_mul(tt[:], t1[:], stat_b["lnw_row"][:])
                        o_sb = pOut.tile([128, D], F32, tag="o", name=f"o{lb}")
                        nc.gpsimd.tensor_add(o_sb[:], tt[:],
                                             stat_b["lnb_row"][:])
                        nc.scalar.dma_start(
                            out_d[lb * 128:(lb + 1) * 128, :], o_sb[:])

    nc.compile()
    return nc


_NC_CACHE = {}


def _get_nc(fold_scale=True, ln_identity=True):
    key = (fold_scale, ln_identity)
    if key not in _NC_CACHE:
        _NC_CACHE[key] = _build_program(*key)
    return _NC_CACHE[key]


def kernel(x, adj, pad_mask, W, b, ln_w, ln_b, edge_weight):
    global LAST_RESULT
    x = np.asarray(x, dtype=np.float32)
    adj = np.asarray(adj, dtype=np.float32)
    pad_mask = np.asarray(pad_mask)
    W = np.asarray(W, dtype=np.float32)
    b = np.asarray(b, dtype=np.float32)
    ln_w = np.asarray(ln_w, dtype=np.float32)
    ln_b = np.asarray(ln_b, dtype=np.float32)
    ew = float(np.asarray(edge_weight).reshape(-1)[0])

    ln_identity = bool(np.all(ln_w == 1.0) and np.all(ln_b == 0.0))
    fold_scale = bool(np.all(b == 0.0) and ew >= 0.0)
    nc = _get_nc(fold_scale, ln_identity)

    ident = np.eye(128, dtype=np.float32)
    # fold_scale: ew commutes with W inside the relu argument, so fold it
    # into the weights host-side; deg/dis stay ew-free.
    w_eff = W.T * ew if fold_scale else W.T
    wt_h = np.ascontiguousarray(w_eff).astype(ml_dtypes.bfloat16)
    eye = np.eye(L, dtype=np.float32)

    in_maps = []
    for c in range(B):
        valid = (~pad_mask[c]).astype(np.float32)
        aT = (adj[c].T * valid[:, None]) * valid[None, :]
        aT += eye
        im = {
            "ident": ident,
            "aT_h": aT.astype(ml_dtypes.bfloat16),
            "xT_h": np.ascontiguousarray(x[c].T).astype(ml_dtypes.bfloat16),
            "x_in": np.ascontiguousarray(x[c]),
            "wt_h": wt_h,
        }
        if not fold_scale:
            im["ewc"] = np.full((128, 1), ew, dtype=np.float32)
            im["b_row"] = np.ascontiguousarray(b.reshape(1, D))
        if not ln_identity:
            im["lnw_row"] = np.ascontiguousarray(ln_w.reshape(1, D))
            im["lnb_row"] = np.ascontiguousarray(ln_b.reshape(1, D))
        in_maps.append(im)

    trace = os.environ.get("KERNEL_TRACE", "0") == "1"
    res = run_bass_kernel_spmd(nc, in_maps, core_ids=list(range(B)), trace=trace)
    LAST_RESULT = res
    out = np.stack([res.results[c]["out_t"] for c in range(B)], axis=0)
    return out


# revision 21
# speedup vs baseline: 2.1371x; 1.4213x over previous
"""GCN layer kernel for TRN2, data-parallel over batch across 8 NeuronCores.

v4: the two matmuls compose linearly (relu sits after BOTH), so
  out2[l,o] = dis_l * sum_j aT[j,l] * (dis_j * (x @ W.T)[j,o])
XW = x @ W.T is computed on the PE during the otherwise idle adjacency
stream window (x tiles are DMA'd first), via PE transposes of x. After deg
-> dis, y = dis * XW (bf16) becomes the moving operand of the single big
matmul with stationary aT column slices, which lands the result directly in
[l, o] layout, one PSUM bank per 128-row block: relu (with the per-partition
dis_l*DSCALE scale) reads PSUM straight, then residual + layernorm. No
second matmul pass, no PSUM->SBUF agg drains, no transposed output fixups.

Phases:
  H: DMA ident, x0, W, x1.., aT0..15. PE: per x tile, 4 transposes ->
     xT bf16, 4 matmuls -> XW psum -> bf16 SBUF; then deg matvecs ride
     the aT stream (PSUM accum over 16 j tiles).
  T: deg rows -> col layout (PE transposes), sqrt off PSUM, reciprocal;
     y[jb] = xw[jb] * dis_col[jb] on DVE/Pool alternating.
  M: 16 i-blocks x (16 j accumulation matmuls), relu+LN chain per block,
     stream out.
"""
import os
import numpy as np
import ml_dtypes

import concourse.bacc as bacc
import concourse.tile as tile
import concourse.mybir as mybir
from concourse.bass_utils import run_bass_kernel_spmd

B, L, D = 8, 2048, 512
JBN = L // 128      # 16 row blocks (j tiles / l blocks)
NCH = L // 512      # 4 deg psum chunks of 512
DBN = D // 128      # 4 d-blocks
LN_EPS = 1e-5
DSCALE = float(D) ** -0.5
F32 = mybir.dt.float32
BF16 = mybir.dt.bfloat16
FP8 = mybir.dt.float8e4
YSCALE = 16.0
MUL = mybir.AluOpType.mult
ADD = mybir.AluOpType.add
SUB = mybir.AluOpType.subtract

LAST_RESULT = None  # BassKernelResults of the most recent run (for profiling)


def _build_program(fold_scale=True, ln_identity=True):
    """fold_scale: edge_weight folded into W host-side and bias == 0, so the
    relu collapses to an activation with per-partition scale dis_l*DSCALE.
    ln_identity: ln_w == 1, ln_b == 0."""
    nc = bacc.Bacc("TRN2", target_bir_lowering=False, debug=False)
    d = {}
    def di(name, shape, dt):
        d[name] = nc.dram_tensor(name, shape, dt, kind="ExternalInput").ap()
    di("ident", [128, 128], F32)
    di("aT_p8", [L // 2, 2 * L], FP8)
    di("xT_h", [D, L], BF16)
    di("x_in", [L, D], F32)
    di("wt_h", [D, D], BF16)
    if not fold_scale:
        di("ewc", [128, 1], F32)
        di("b_row", [1, D], F32)
    if not ln_identity:
        di("lnw_row", [1, D], F32)
        di("lnb_row", [1, D], F32)
    out_d = nc.dram_tensor("out_t", [L, D], F32, kind="ExternalOutput").ap()

    with tile.TileContext(nc) as tc:
        with tc.tile_pool(name="pA", bufs=JBN) as pA, \
             tc.tile_pool(name="pX", bufs=JBN) as pX, \
             tc.tile_pool(name="pY", bufs=JBN) as pY, \
             tc.tile_pool(name="pXW", bufs=JBN) as pXW, \
             tc.tile_pool(name="pW", bufs=DBN) as pW, \
             tc.tile_pool(name="pCol", bufs=12) as pCol, \
             tc.tile_pool(name="pSmall", bufs=1) as pSmall:

            # ---- persistent arrays ----
            aT_t = [pA.tile([128, 2, L], FP8, tag="aT", name=f"aT{j}")
                    for j in range(JBN // 2)]
            x_t = [pX.tile([128, D], F32, tag="x", name=f"x{j}") for j in range(JBN)]
            xw_t = [pXW.tile([128, D], BF16, tag="xw", name=f"xw{j}")
                    for j in range(JBN)]
            pXTh_cm = tc.tile_pool(name="pXTh", bufs=DBN)
            pXTh = pXTh_cm.__enter__()
            xTh_t = [pXTh.tile([128, L], BF16, tag="xTh", name=f"xTh{m}")
                     for m in range(DBN)]
            y_t = [pY.tile([128, 2, D], FP8, tag="y", name=f"y{j}")
                   for j in range(JBN // 2)]
            wt_t = [pW.tile([128, D], BF16, tag="wt", name=f"wt{k}")
                    for k in range(DBN)]
            eps_t = pSmall.tile([128, 1], F32, tag="eps")
            nc.vector.memset(eps_t[:], LN_EPS)
            # dual-fp8 ldweights needs 16B-aligned kk stride: 16 duplicate
            # ones columns; rows 1..15 of each deg psum output are ignored
            onesc_t = pSmall.tile([128, 2, 16], FP8, tag="onesc")
            nc.vector.memset(onesc_t[:], 1.0)
            # touch every activation function now so the Act table loads
            # happen during the DMA stream, not on the critical path later
            warm_t = pSmall.tile([128, 1], F32, tag="warm")
            nc.scalar.sqrt(warm_t[:], eps_t[:])
            nc.scalar.activation(warm_t[:], eps_t[:],
                                 mybir.ActivationFunctionType.Square)
            nc.scalar.activation(warm_t[:], eps_t[:],
                                 mybir.ActivationFunctionType.Relu)
            stat_b = {}

            # DMA order: ident, W, xT (for XW), aT (gates deg), x f32 last
            for k in range(DBN):
                nc.sync.dma_start(wt_t[k][:], d["wt_h"][k * 128:(k + 1) * 128, :])
                rsl = slice(k * 128, (k + 1) * 128)
                if k < 2:
                    # split: the first 512 columns feed XW quarter 0 sooner
                    nc.sync.dma_start(xTh_t[k][:, 0:512], d["xT_h"][rsl, 0:512])
                    nc.sync.dma_start(xTh_t[k][:, 512:L], d["xT_h"][rsl, 512:L])
                else:
                    nc.sync.dma_start(xTh_t[k][:], d["xT_h"][rsl, :])
            ident_t = pSmall.tile([128, 128], F32, tag="ident")
            nc.sync.dma_start(ident_t[:], d["ident"][:])
            for j2 in range(JBN // 2):
                nc.sync.dma_start(aT_t[j2][:],
                                  d["aT_p8"][j2 * 128:(j2 + 1) * 128, :])
            for jb in range(JBN):
                nc.sync.dma_start(x_t[jb][:], d["x_in"][jb * 128:(jb + 1) * 128, :])
            if not fold_scale:
                ewc_t = pSmall.tile([128, 1], F32, tag="ew")
                nc.scalar.dma_start(ewc_t[:], d["ewc"][:])
                b_r = pSmall.tile([1, D], F32, tag="brow")
                nc.scalar.dma_start(b_r[:], d["b_row"][:])
                bbT = pSmall.tile([128, D], F32, tag="bb")
                nc.gpsimd.partition_broadcast(bbT[:], b_r[:])
            if not ln_identity:
                rows = {}
                for nm in ("lnw_row", "lnb_row"):
                    r = pSmall.tile([1, D], F32, tag=nm, name=nm + "_t")
                    nc.scalar.dma_start(r[:], d[nm][:])
                    rows[nm] = r
                for nm in ("lnw_row", "lnb_row"):
                    t = pSmall.tile([128, D], F32, tag=nm + "b", name=nm + "_b")
                    nc.gpsimd.partition_broadcast(t[:], rows[nm][:])
                    stat_b[nm] = t

            # ---- phase H: XW = x @ W.T on PE from the host-transposed x,
            # m-major over jb-halves in 8 PSUM banks, while aT streams ----
            # XW emission is interleaved with the deg matvecs: quarter 0
            # runs before the aT stream lands; the remaining 48 XW matmuls
            # are doled out 3 per deg tile so the PE rides the stream with
            # no idle (per-tile slot: 4 deg matvecs + 3 XW matmuls ~= the
            # 1.46us tile DMA cadence).
            psXW_cm = tc.tile_pool(name="psXW", bufs=3, space="PSUM")
            psXW = psXW_cm.__enter__()

            def xw_instruction_stream():
                for q in range(4):
                    jbs = range(q * 4, q * 4 + 4)
                    xwp = {jb: psXW.tile([128, D], F32, tag="xwp",
                                         name=f"xwp{jb}") for jb in jbs}
                    for m in range(DBN):
                        for jb in jbs:
                            yield "mm", (lambda jb=jb, m=m, xwp=xwp:
                                nc.tensor.matmul(
                                    xwp[jb][:],
                                    xTh_t[m][:, jb * 128:(jb + 1) * 128],
                                    wt_t[m][:],
                                    start=(m == 0), stop=(m == DBN - 1)))
                    for jb in jbs:
                        if jb % 2 == 0:
                            yield "drain", (lambda jb=jb, xwp=xwp:
                                nc.vector.tensor_copy(xw_t[jb][:], xwp[jb][:]))
                        else:
                            yield "drain", (lambda jb=jb, xwp=xwp:
                                nc.scalar.copy(xw_t[jb][:], xwp[jb][:]))

            xw_stream = xw_instruction_stream()
            def emit_xw(k):
                n = 0
                for kind, op in xw_stream:
                    op()
                    if kind == "mm":
                        n += 1
                        if n >= k:
                            break

            emit_xw(20)  # quarter 0 + the head of quarter 1

            with tc.tile_pool(name="psDeg", bufs=4, space="PSUM") as psDeg, \
                 tc.tile_pool(name="psPT", bufs=1, space="PSUM") as psPT, \
                 tc.tile_pool(name="pTr", bufs=1) as pTr:
                # deg: 4 chunk accumulators, DoubleRow dst must start at
                # partition 0 so each chunk gets its own bank (row 0 used)
                deg_ps = [psDeg.tile([128, 512], F32, tag="deg",
                                     name=f"deg_ps{i}") for i in range(NCH)]
                for j2 in range(JBN // 2):
                    for n in range(NCH):
                        nc.tensor.matmul(
                            deg_ps[n][0:16, :],
                            onesc_t[:],
                            aT_t[j2][:, :, n * 512:(n + 1) * 512],
                            start=(j2 == 0), stop=(j2 == JBN // 2 - 1),
                            perf_mode=mybir.MatmulPerfMode.DoubleRow)
                    emit_xw(6)
                emit_xw(1000)  # drain any remaining XW work
                # ---- phase T: deg -> dis -> y, two-stage pipeline ----
                r_sb = pTr.tile([128, 1024], F32, tag="rsb")
                rc_ps = psPT.tile([128, JBN], F32, tag="rc")
                std_col = pCol.tile([128, JBN], F32, tag="stdc", bufs=1)
                dis_col = pCol.tile([128, JBN], F32, tag="disc", bufs=1)
                diss_col = pCol.tile([128, JBN], F32, tag="dissc", bufs=1)
                for t in range(2):
                    csl = slice(t * 8, t * 8 + 8)
                    eng_copy = (nc.vector.tensor_copy if t == 0
                                else nc.scalar.copy)
                    eng_copy(r_sb[0:1, t * 512:(t + 1) * 512],
                             deg_ps[2 * t][0:1, :])
                    eng_copy(r_sb[32:33, t * 512:(t + 1) * 512],
                             deg_ps[2 * t + 1][0:1, :])
                    for v in range(t * 8, t * 8 + 8):
                        n, c = v // 4, v % 4
                        po = 32 * (n % 2)
                        fo = (n // 2) * 512 + c * 128
                        nc.tensor.transpose(
                            rc_ps[:, v:v + 1],
                            r_sb[po:po + 1, fo:fo + 128],
                            ident_t[po:po + 1, po:po + 1])
                    nc.scalar.sqrt(std_col[:, csl], rc_ps[:, csl])
                    nc.vector.reciprocal(dis_col[:, csl], std_col[:, csl])
                    # y = YSCALE * dis * xw keeps fp8 out of the subnormals;
                    # the 1/YSCALE rides the relu scale (c1s)
                    nc.scalar.mul(diss_col[:, csl], dis_col[:, csl], YSCALE)
                    for jb in range(t * 8, t * 8 + 8):
                        ysl = y_t[jb // 2][:, jb % 2, :]
                        if jb % 2 == 0:
                            nc.vector.tensor_scalar_mul(
                                ysl, xw_t[jb][:], diss_col[:, jb:jb + 1])
                        else:
                            nc.scalar.mul(ysl, xw_t[jb][:],
                                          diss_col[:, jb:jb + 1])

            psXW_cm.__exit__(None, None, None)
            pXTh_cm.__exit__(None, None, None)

            if fold_scale:
                c1s_col = pCol.tile([128, JBN], F32, tag="c1s", bufs=1)
                nc.scalar.mul(c1s_col[:], dis_col[:], DSCALE / YSCALE)
            else:
                c1_col = pCol.tile([128, JBN], F32, tag="c1c", bufs=1)
                nc.vector.tensor_scalar_mul(c1_col[:], dis_col[:], ewc_t[:])
                nc.scalar.mul(c1_col[:], c1_col[:], 1.0 / YSCALE)

            # ---- phase M: one matmul group + relu/LN chain per 128-row
            # block; 8 PSUM banks rotate, freed by the relu read ----
            with tc.tile_pool(name="psMM", bufs=8, space="PSUM") as psMM, \
                 tc.tile_pool(name="pScr", bufs=9) as pScr, \
                 tc.tile_pool(name="pOut", bufs=5) as pOut:
                for ib in range(JBN):
                    lb = ib
                    ps = psMM.tile([128, 512], F32, tag="mm", name=f"mm{ib}")
                    # last block: split into o-halves so half-a's relu/hh/
                    # stats overlap half-b's matmuls, shortening the tail
                    halves = ((slice(0, 512),),) if ib != JBN - 1 else \
                        ((slice(0, 256),), (slice(256, 512),))
                    r = pScr.tile([128, D], F32, tag="scr", name=f"r{lb}")
                    hh = pScr.tile([128, D], F32, tag="scr", name=f"hh{lb}")
                    st6 = pCol.tile([128, len(halves), 6], F32, tag="lnst",
                                    name=f"st{lb}")
                    for hi, (osl,) in enumerate(halves):
                        for j2 in range(JBN // 2):
                            nc.tensor.matmul(
                                ps[:, osl],
                                aT_t[j2][:, :, ib * 128:(ib + 1) * 128],
                                y_t[j2][:, :, osl], start=(j2 == 0),
                                stop=(j2 == JBN // 2 - 1),
                                perf_mode=mybir.MatmulPerfMode.DoubleRow)
                        if fold_scale:
                            nc.scalar.activation(
                                r[:, osl], ps[:, osl],
                                mybir.ActivationFunctionType.Relu,
                                scale=c1s_col[:, lb:lb + 1])
                        else:
                            tmp = pScr.tile([128, D], F32, tag="scr",
                                            name=f"tb{lb}_{hi}")
                            nc.vector.scalar_tensor_tensor(
                                tmp[:, osl], ps[:, osl], c1_col[:, lb:lb + 1],
                                bbT[:, osl], MUL, ADD)
                            nc.scalar.activation(
                                r[:, osl], tmp[:, osl],
                                mybir.ActivationFunctionType.Relu,
                                scale=DSCALE)
                        heng = nc.vector if lb % 2 == 0 else nc.gpsimd
                        heng.tensor_add(hh[:, osl], r[:, osl],
                                        x_t[lb][:, osl])
                        nc.vector.bn_stats(st6[:, hi, :], hh[:, osl])
                    mv = pCol.tile([128, 2], F32, tag="lnmv", name=f"mv{lb}")
                    nc.vector.bn_aggr(mv[:], st6[:])
                    stdt = pCol.tile([128, 1], F32, tag="lncol", name=f"sd{lb}")
                    nc.scalar.activation(
                        stdt[:], mv[:, 1:2], mybir.ActivationFunctionType.Sqrt,
                        bias=eps_t[:])
                    rstd = pCol.tile([128, 1], F32, tag="lncol", name=f"rs{lb}")
                    nc.vector.reciprocal(rstd[:], stdt[:])
                    eng1 = nc.gpsimd if lb % 2 == 0 else nc.vector
                    t1 = pOut.tile([128, D], F32, tag="o", name=f"t1{lb}")
                    eng1.tensor_scalar(t1[:], hh[:], mv[:, 0:1], rstd[:],
                                       SUB, MUL)
                    if ln_identity:
                        nc.scalar.dma_start(
                            out_d[lb * 128:(lb + 1) * 128, :], t1[:])
                    else:
                        tt = pScr.tile([128, D], F32, tag="scr", name=f"tt{lb}")
                        teng = nc.vector if lb % 2 == 0 else nc.gpsimd
                        teng.tensor_mul(tt[:], t1[:], stat_b["lnw_row"][:])
                        o_sb = pOut.tile([128, D], F32, tag="o", name=f"o{lb}")
                        nc.gpsimd.tensor_add(o_sb[:], tt[:],
                                             stat_b["lnb_row"][:])
                        nc.scalar.dma_start(
                            out_d[lb * 128:(lb + 1) * 128, :], o_sb[:])

    nc.compile()
    return nc


_NC_CACHE = {}


def _get_nc(fold_scale=True, ln_identity=True):
    key = (fold_scale, ln_identity)
    if key not in _NC_CACHE:
        _NC_CACHE[key] = _build_program(*key)
    return _NC_CACHE[key]


def kernel(x, adj, pad_mask, W, b, ln_w, ln_b, edge_weight):
    global LAST_RESULT
    x = np.asarray(x, dtype=np.float32)
    adj = np.asarray(adj, dtype=np.float32)
    pad_mask = np.asarray(pad_mask)
    W = np.asarray(W, dtype=np.float32)
    b = np.asarray(b, dtype=np.float32)
    ln_w = np.asarray(ln_w, dtype=np.float32)
    ln_b = np.asarray(ln_b, dtype=np.float32)
    ew = float(np.asarray(edge_weight).reshape(-1)[0])

    ln_identity = bool(np.all(ln_w == 1.0) and np.all(ln_b == 0.0))
    fold_scale = bool(np.all(b == 0.0) and ew >= 0.0)
    nc = _get_nc(fold_scale, ln_identity)

    ident = np.eye(128, dtype=np.float32)
    # fold_scale: ew commutes with W inside the relu argument, so fold it
    # into the weights host-side; deg/dis stay ew-free.
    w_eff = W.T * ew if fold_scale else W.T
    wt_h = np.ascontiguousarray(w_eff).astype(ml_dtypes.bfloat16)
    eye = np.eye(L, dtype=np.float32)

    in_maps = []
    for c in range(B):
        valid = (~pad_mask[c]).astype(np.float32)
        aT = (adj[c].T * valid[:, None]) * valid[None, :]
        aT += eye
        aT8 = aT.astype(ml_dtypes.float8_e4m3)
        aT8 = np.ascontiguousarray(
            aT8.reshape(8, 2, 128, L).transpose(0, 2, 1, 3).reshape(L // 2, 2 * L))
        im = {
            "ident": ident,
            "aT_p8": aT8,
            "xT_h": np.ascontiguousarray(x[c].T).astype(ml_dtypes.bfloat16),
            "x_in": np.ascontiguousarray(x[c]),
            "wt_h": wt_h,
        }
        if not fold_scale:
            im["ewc"] = np.full((128, 1), ew, dtype=np.float32)
            im["b_row"] = np.ascontiguousarray(b.reshape(1, D))
        if not ln_identity:
            im["lnw_row"] = np.ascontiguousarray(ln_w.reshape(1, D))
            im["lnb_row"] = np.ascontiguousarray(ln_b.reshape(1, D))
        in_maps.append(im)

    trace = os.environ.get("KERNEL_TRACE", "0") == "1"
    res = run_bass_kernel_spmd(nc, in_maps, core_ids=list(range(B)), trace=trace)
    LAST_RESULT = res
    out = np.stack([res.results[c]["out_t"] for c in range(B)], axis=0)
    return out


# revision 23
# speedup vs baseline: 2.2713x; 1.0628x over previous
"""GCN layer kernel for TRN2, data-parallel over batch across 8 NeuronCores.

The two matmuls of the layer compose linearly (the relu sits after both):
  out2[l,o] = dis_l * sum_j aT[j,l] * dis_j * (x @ W.T)[j,o]
so the device computes XW = x @ W.T first (bf16, from a host-transposed x,
during the adjacency DMA stream), then one big fp8 DoubleRow matmul against
the mask+self-loop-folded adjacency (host-prepared, fp8, paired-row layout
[128, 2, L] per j-tile) lands the pre-relu directly in [l, o] layout.

Phases:
  H: DMA wt/xT (bf16), then aT (fp8), then x (f32). PE: XW quarters
     (m-major, 3 PSUM banks) interleaved 3-matmuls-per-tile with the deg
     row-sum matvecs (fp8 DoubleRow vs a 16-wide ones stationary) so the PE
     rides the aT stream without idling.
  T: deg rows -> column layout via PE transposes, sqrt off PSUM,
     reciprocal; y = 16 * dis * XW cast to fp8 (the 16 keeps fp8 out of
     subnormals; 1/16 rides the relu scale).
  M: per 128-row block: 8 DoubleRow accumulation matmuls (j pairs), relu
     straight off PSUM with per-partition scale dis_l/(16*sqrt(D)), bf16
     residual hh = relu + x, bn_stats/bn_aggr moments, normalize, stream
     out. The last block runs in o-halves to shorten the serial tail.

Precision: adjacency and y in fp8e4m3, XW/W in bf16, accumulation in fp32
PSUM, LN in fp32 with bf16 hh. Measured rel err ~2-5e-3 vs the fp32
reference (gate: 2e-2).
"""
import os
import numpy as np
import ml_dtypes

import concourse.bacc as bacc
import concourse.tile as tile
import concourse.mybir as mybir
from concourse.bass_utils import run_bass_kernel_spmd

B, L, D = 8, 2048, 512
JBN = L // 128      # 16 row blocks (j tiles / l blocks)
NCH = L // 512      # 4 deg psum chunks of 512
DBN = D // 128      # 4 d-blocks
LN_EPS = 1e-5
DSCALE = float(D) ** -0.5
F32 = mybir.dt.float32
BF16 = mybir.dt.bfloat16
FP8 = mybir.dt.float8e4
YSCALE = 16.0
MUL = mybir.AluOpType.mult
ADD = mybir.AluOpType.add
SUB = mybir.AluOpType.subtract

LAST_RESULT = None  # BassKernelResults of the most recent run (for profiling)


def _build_program(fold_scale=True, ln_identity=True):
    """fold_scale: edge_weight folded into W host-side and bias == 0, so the
    relu collapses to an activation with per-partition scale dis_l*DSCALE.
    ln_identity: ln_w == 1, ln_b == 0."""
    nc = bacc.Bacc("TRN2", target_bir_lowering=False, debug=False)
    d = {}
    def di(name, shape, dt):
        d[name] = nc.dram_tensor(name, shape, dt, kind="ExternalInput").ap()
    di("ident", [128, 128], F32)
    di("aT_p8", [L // 2, 2 * L], FP8)
    di("xT_h", [D, L], BF16)
    di("x_in", [L, D], F32)
    di("wt_h", [D, D], BF16)
    if not fold_scale:
        di("ewc", [128, 1], F32)
        di("b_row", [1, D], F32)
    if not ln_identity:
        di("lnw_row", [1, D], F32)
        di("lnb_row", [1, D], F32)
    out_d = nc.dram_tensor("out_t", [L, D], F32, kind="ExternalOutput").ap()

    with tile.TileContext(nc) as tc:
        with tc.tile_pool(name="pA", bufs=JBN) as pA, \
             tc.tile_pool(name="pX", bufs=JBN) as pX, \
             tc.tile_pool(name="pY", bufs=JBN) as pY, \
             tc.tile_pool(name="pXW", bufs=JBN) as pXW, \
             tc.tile_pool(name="pW", bufs=DBN) as pW, \
             tc.tile_pool(name="pCol", bufs=12) as pCol, \
             tc.tile_pool(name="pSmall", bufs=1) as pSmall:

            # ---- persistent arrays ----
            aT_t = [pA.tile([128, 2, L], FP8, tag="aT", name=f"aT{j}")
                    for j in range(JBN // 2)]
            x_t = [pX.tile([128, D], F32, tag="x", name=f"x{j}") for j in range(JBN)]
            xw_t = [pXW.tile([128, D], BF16, tag="xw", name=f"xw{j}")
                    for j in range(JBN)]
            pXTh_cm = tc.tile_pool(name="pXTh", bufs=DBN)
            pXTh = pXTh_cm.__enter__()
            xTh_t = [pXTh.tile([128, L], BF16, tag="xTh", name=f"xTh{m}")
                     for m in range(DBN)]
            y_t = [pY.tile([128, 2, D], FP8, tag="y", name=f"y{j}")
                   for j in range(JBN // 2)]
            wt_t = [pW.tile([128, D], BF16, tag="wt", name=f"wt{k}")
                    for k in range(DBN)]
            eps_t = pSmall.tile([128, 1], F32, tag="eps")
            nc.vector.memset(eps_t[:], LN_EPS)
            # dual-fp8 ldweights needs 16B-aligned kk stride: 16 duplicate
            # ones columns; rows 1..15 of each deg psum output are ignored
            onesc_t = pSmall.tile([128, 2, 16], FP8, tag="onesc")
            nc.vector.memset(onesc_t[:], 1.0)
            # touch every activation function now so the Act table loads
            # happen during the DMA stream, not on the critical path later
            warm_t = pSmall.tile([128, 1], F32, tag="warm")
            nc.scalar.sqrt(warm_t[:], eps_t[:])
            nc.scalar.activation(warm_t[:], eps_t[:],
                                 mybir.ActivationFunctionType.Square)
            nc.scalar.activation(warm_t[:], eps_t[:],
                                 mybir.ActivationFunctionType.Relu)
            stat_b = {}

            # DMA order: ident, W, xT (for XW), aT (gates deg), x f32 last
            for k in range(DBN):
                nc.sync.dma_start(wt_t[k][:], d["wt_h"][k * 128:(k + 1) * 128, :])
                rsl = slice(k * 128, (k + 1) * 128)
                if k < 2:
                    # split: the first 512 columns feed XW quarter 0 sooner
                    nc.sync.dma_start(xTh_t[k][:, 0:512], d["xT_h"][rsl, 0:512])
                    nc.sync.dma_start(xTh_t[k][:, 512:L], d["xT_h"][rsl, 512:L])
                else:
                    nc.sync.dma_start(xTh_t[k][:], d["xT_h"][rsl, :])
            ident_t = pSmall.tile([128, 128], F32, tag="ident")
            nc.sync.dma_start(ident_t[:], d["ident"][:])
            for j2 in range(JBN // 2):
                nc.sync.dma_start(aT_t[j2][:],
                                  d["aT_p8"][j2 * 128:(j2 + 1) * 128, :])
            for jb in range(JBN):
                nc.sync.dma_start(x_t[jb][:], d["x_in"][jb * 128:(jb + 1) * 128, :])
            if not fold_scale:
                ewc_t = pSmall.tile([128, 1], F32, tag="ew")
                nc.scalar.dma_start(ewc_t[:], d["ewc"][:])
                b_r = pSmall.tile([1, D], F32, tag="brow")
                nc.scalar.dma_start(b_r[:], d["b_row"][:])
                bbT = pSmall.tile([128, D], F32, tag="bb")
                nc.gpsimd.partition_broadcast(bbT[:], b_r[:])
            if not ln_identity:
                rows = {}
                for nm in ("lnw_row", "lnb_row"):
                    r = pSmall.tile([1, D], F32, tag=nm, name=nm + "_t")
                    nc.scalar.dma_start(r[:], d[nm][:])
                    rows[nm] = r
                for nm in ("lnw_row", "lnb_row"):
                    t = pSmall.tile([128, D], F32, tag=nm + "b", name=nm + "_b")
                    nc.gpsimd.partition_broadcast(t[:], rows[nm][:])
                    stat_b[nm] = t

            # ---- phase H: XW = x @ W.T on PE from the host-transposed x,
            # m-major over jb-halves in 8 PSUM banks, while aT streams ----
            # XW emission is interleaved with the deg matvecs: quarter 0
            # runs before the aT stream lands; the remaining 48 XW matmuls
            # are doled out 3 per deg tile so the PE rides the stream with
            # no idle (per-tile slot: 4 deg matvecs + 3 XW matmuls ~= the
            # 1.46us tile DMA cadence).
            psXW_cm = tc.tile_pool(name="psXW", bufs=3, space="PSUM")
            psXW = psXW_cm.__enter__()

            def xw_instruction_stream():
                for q in range(4):
                    jbs = range(q * 4, q * 4 + 4)
                    xwp = {jb: psXW.tile([128, D], F32, tag="xwp",
                                         name=f"xwp{jb}") for jb in jbs}
                    for m in range(DBN):
                        for jb in jbs:
                            yield "mm", (lambda jb=jb, m=m, xwp=xwp:
                                nc.tensor.matmul(
                                    xwp[jb][:],
                                    xTh_t[m][:, jb * 128:(jb + 1) * 128],
                                    wt_t[m][:],
                                    start=(m == 0), stop=(m == DBN - 1)))
                    for jb in jbs:
                        if jb % 2 == 0:
                            yield "drain", (lambda jb=jb, xwp=xwp:
                                nc.vector.tensor_copy(xw_t[jb][:], xwp[jb][:]))
                        else:
                            yield "drain", (lambda jb=jb, xwp=xwp:
                                nc.scalar.copy(xw_t[jb][:], xwp[jb][:]))

            xw_stream = xw_instruction_stream()
            def emit_xw(k):
                n = 0
                for kind, op in xw_stream:
                    op()
                    if kind == "mm":
                        n += 1
                        if n >= k:
                            break

            emit_xw(20)  # quarter 0 + the head of quarter 1

            with tc.tile_pool(name="psDeg", bufs=4, space="PSUM") as psDeg, \
                 tc.tile_pool(name="psPT", bufs=1, space="PSUM") as psPT, \
                 tc.tile_pool(name="pTr", bufs=1) as pTr:
                # deg: 4 chunk accumulators, DoubleRow dst must start at
                # partition 0 so each chunk gets its own bank (row 0 used)
                deg_ps = [psDeg.tile([128, 512], F32, tag="deg",
                                     name=f"deg_ps{i}") for i in range(NCH)]
                for j2 in range(JBN // 2):
                    for n in range(NCH):
                        nc.tensor.matmul(
                            deg_ps[n][0:16, :],
                            onesc_t[:],
                            aT_t[j2][:, :, n * 512:(n + 1) * 512],
                            start=(j2 == 0), stop=(j2 == JBN // 2 - 1),
                            perf_mode=mybir.MatmulPerfMode.DoubleRow)
                    emit_xw(6)
                emit_xw(1000)  # drain any remaining XW work
                # ---- phase T: deg -> dis -> y, two-stage pipeline ----
                r_sb = pTr.tile([128, 1024], F32, tag="rsb")
                rc_ps = psPT.tile([128, JBN], F32, tag="rc")
                std_col = pCol.tile([128, JBN], F32, tag="stdc", bufs=1)
                dis_col = pCol.tile([128, JBN], F32, tag="disc", bufs=1)
                diss_col = pCol.tile([128, JBN], F32, tag="dissc", bufs=1)
                for t in range(2):
                    csl = slice(t * 8, t * 8 + 8)
                    eng_copy = (nc.vector.tensor_copy if t == 0
                                else nc.scalar.copy)
                    eng_copy(r_sb[0:1, t * 512:(t + 1) * 512],
                             deg_ps[2 * t][0:1, :])
                    eng_copy(r_sb[32:33, t * 512:(t + 1) * 512],
                             deg_ps[2 * t + 1][0:1, :])
                    for v in range(t * 8, t * 8 + 8):
                        n, c = v // 4, v % 4
                        po = 32 * (n % 2)
                        fo = (n // 2) * 512 + c * 128
                        nc.tensor.transpose(
                            rc_ps[:, v:v + 1],
                            r_sb[po:po + 1, fo:fo + 128],
                            ident_t[po:po + 1, po:po + 1])
                    nc.scalar.sqrt(std_col[:, csl], rc_ps[:, csl])
                    nc.vector.reciprocal(dis_col[:, csl], std_col[:, csl])
                    # y = YSCALE * dis * xw keeps fp8 out of the subnormals;
                    # the 1/YSCALE rides the relu scale (c1s)
                    nc.scalar.mul(diss_col[:, csl], dis_col[:, csl], YSCALE)
                    for jb in range(t * 8, t * 8 + 8):
                        ysl = y_t[jb // 2][:, jb % 2, :]
                        if jb % 2 == 0:
                            nc.vector.tensor_scalar_mul(
                                ysl, xw_t[jb][:], diss_col[:, jb:jb + 1])
                        else:
                            nc.scalar.mul(ysl, xw_t[jb][:],
                                          diss_col[:, jb:jb + 1])

            psXW_cm.__exit__(None, None, None)
            pXTh_cm.__exit__(None, None, None)

            if fold_scale:
                c1s_col = pCol.tile([128, JBN], F32, tag="c1s", bufs=1)
                nc.scalar.mul(c1s_col[:], dis_col[:], DSCALE / YSCALE)
            else:
                c1_col = pCol.tile([128, JBN], F32, tag="c1c", bufs=1)
                nc.vector.tensor_scalar_mul(c1_col[:], dis_col[:], ewc_t[:])
                nc.scalar.mul(c1_col[:], c1_col[:], 1.0 / YSCALE)

            # ---- phase M: one matmul group + relu/LN chain per 128-row
            # block; 8 PSUM banks rotate, freed by the relu read ----
            with tc.tile_pool(name="psMM", bufs=8, space="PSUM") as psMM, \
                 tc.tile_pool(name="pScr", bufs=9) as pScr, \
                 tc.tile_pool(name="pOut", bufs=5) as pOut:
                for ib in range(JBN):
                    lb = ib
                    ps = psMM.tile([128, 512], F32, tag="mm", name=f"mm{ib}")
                    # last block: split into o-halves so half-a's relu/hh/
                    # stats overlap half-b's matmuls, shortening the tail
                    halves = ((slice(0, 512),),) if ib != JBN - 1 else \
                        ((slice(0, 256),), (slice(256, 512),))
                    r = pScr.tile([128, D], F32, tag="scr", name=f"r{lb}")
                    # bf16 hh: halves the DVE/Pool cost of hh/bn_stats/t1
                    hh = pScr.tile([128, D], BF16, tag="scrh", name=f"hh{lb}")
                    st6 = pCol.tile([128, len(halves), 6], F32, tag="lnst",
                                    name=f"st{lb}")
                    for hi, (osl,) in enumerate(halves):
                        for j2 in range(JBN // 2):
                            nc.tensor.matmul(
                                ps[:, osl],
                                aT_t[j2][:, :, ib * 128:(ib + 1) * 128],
                                y_t[j2][:, :, osl], start=(j2 == 0),
                                stop=(j2 == JBN // 2 - 1),
                                perf_mode=mybir.MatmulPerfMode.DoubleRow)
                        if fold_scale:
                            nc.scalar.activation(
                                r[:, osl], ps[:, osl],
                                mybir.ActivationFunctionType.Relu,
                                scale=c1s_col[:, lb:lb + 1])
                        else:
                            tmp = pScr.tile([128, D], F32, tag="scr",
                                            name=f"tb{lb}_{hi}")
                            nc.vector.scalar_tensor_tensor(
                                tmp[:, osl], ps[:, osl], c1_col[:, lb:lb + 1],
                                bbT[:, osl], MUL, ADD)
                            nc.scalar.activation(
                                r[:, osl], tmp[:, osl],
                                mybir.ActivationFunctionType.Relu,
                                scale=DSCALE)
                        heng = nc.vector if lb % 2 == 0 else nc.gpsimd
                        heng.tensor_add(hh[:, osl], r[:, osl],
                                        x_t[lb][:, osl])
                        nc.vector.bn_stats(st6[:, hi, :], hh[:, osl])
                    mv = pCol.tile([128, 2], F32, tag="lnmv", name=f"mv{lb}")
                    nc.vector.bn_aggr(mv[:], st6[:])
                    stdt = pCol.tile([128, 1], F32, tag="lncol", name=f"sd{lb}")
                    nc.scalar.activation(
                        stdt[:], mv[:, 1:2], mybir.ActivationFunctionType.Sqrt,
                        bias=eps_t[:])
                    rstd = pCol.tile([128, 1], F32, tag="lncol", name=f"rs{lb}")
                    nc.vector.reciprocal(rstd[:], stdt[:])
                    eng1 = nc.gpsimd if lb % 2 == 0 else nc.vector
                    t1 = pOut.tile([128, D], F32, tag="o", name=f"t1{lb}")
                    eng1.tensor_scalar(t1[:], hh[:], mv[:, 0:1], rstd[:],
                                       SUB, MUL)
                    if ln_identity:
                        nc.scalar.dma_start(
                            out_d[lb * 128:(lb + 1) * 128, :], t1[:])
                    else:
                        tt = pScr.tile([128, D], F32, tag="scr", name=f"tt{lb}")
                        teng = nc.vector if lb % 2 == 0 else nc.gpsimd
                        teng.tensor_mul(tt[:], t1[:], stat_b["lnw_row"][:])
                        o_sb = pOut.tile([128, D], F32, tag="o", name=f"o{lb}")
                        nc.gpsimd.tensor_add(o_sb[:], tt[:],
                                             stat_b["lnb_row"][:])
                        nc.scalar.dma_start(
                            out_d[lb * 128:(lb + 1) * 128, :], o_sb[:])

    nc.compile()
    return nc


_NC_CACHE = {}


def _get_nc(fold_scale=True, ln_identity=True):
    key = (fold_scale, ln_identity)
    if key not in _NC_CACHE:
        _NC_CACHE[key] = _build_program(*key)
    return _NC_CACHE[key]


def kernel(x, adj, pad_mask, W, b, ln_w, ln_b, edge_weight):
    global LAST_RESULT
    x = np.asarray(x, dtype=np.float32)
    adj = np.asarray(adj, dtype=np.float32)
    pad_mask = np.asarray(pad_mask)
    W = np.asarray(W, dtype=np.float32)
    b = np.asarray(b, dtype=np.float32)
    ln_w = np.asarray(ln_w, dtype=np.float32)
    ln_b = np.asarray(ln_b, dtype=np.float32)
    ew = float(np.asarray(edge_weight).reshape(-1)[0])

    ln_identity = bool(np.all(ln_w == 1.0) and np.all(ln_b == 0.0))
    fold_scale = bool(np.all(b == 0.0) and ew >= 0.0)
    nc = _get_nc(fold_scale, ln_identity)

    ident = np.eye(128, dtype=np.float32)
    # fold_scale: ew commutes with W inside the relu argument, so fold it
    # into the weights host-side; deg/dis stay ew-free.
    w_eff = W.T * ew if fold_scale else W.T
    wt_h = np.ascontiguousarray(w_eff).astype(ml_dtypes.bfloat16)
    eye = np.eye(L, dtype=np.float32)

    in_maps = []
    for c in range(B):
        valid = (~pad_mask[c]).astype(np.float32)
        aT = (adj[c].T * valid[:, None]) * valid[None, :]
        aT += eye
        aT8 = aT.astype(ml_dtypes.float8_e4m3)
        aT8 = np.ascontiguousarray(
            aT8.reshape(8, 2, 128, L).transpose(0, 2, 1, 3).reshape(L // 2, 2 * L))
        im = {
            "ident": ident,
            "aT_p8": aT8,
            "xT_h": np.ascontiguousarray(x[c].T).astype(ml_dtypes.bfloat16),
            "x_in": np.ascontiguousarray(x[c]),
            "wt_h": wt_h,
        }
        if not fold_scale:
            im["ewc"] = np.full((128, 1), ew, dtype=np.float32)
            im["b_row"] = np.ascontiguousarray(b.reshape(1, D))
        if not ln_identity:
            im["lnw_row"] = np.ascontiguousarray(ln_w.reshape(1, D))
            im["lnb_row"] = np.ascontiguousarray(ln_b.reshape(1, D))
        in_maps.append(im)

    trace = os.environ.get("KERNEL_TRACE", "0") == "1"
    res = run_bass_kernel_spmd(nc, in_maps, core_ids=list(range(B)), trace=trace)
    LAST_RESULT = res
    out = np.stack([res.results[c]["out_t"] for c in range(B)], axis=0)
    return out
